# revision 1
# baseline (speedup 1.0000x reference)
"""MultiHeadAttention (QK-RMSNorm + RoPE + causal) Trainium2 Bass kernel.

Sharding: 8 cores = 2 batches x 4 head-groups (4 heads each).
Each core computes a partial (2048, 1024) output (its heads' contribution
through the output projection); host sums the 4 group-partials per batch.

SPMD: all cores run the identical program on different input slices.
"""

import math
import sys
from contextlib import ExitStack

import numpy as np

sys.path.insert(0, "/opt/trn_rl_repo")

import concourse.bass as bass  # noqa: E402
import concourse.bacc as bacc  # noqa: E402
import concourse.tile as tile  # noqa: E402
from concourse import mybir  # noqa: E402

B = 2
T = 2048
D = 1024
H = 16
HD = 64
G = 4  # heads per core
NCORES = 8
NT = T // 128  # 16 t-tiles
ND = D // 128  # 8 d-chunks
EPS = 1e-6
ROPE_BASE = 10000.0
MASK_NEG = -30000.0

F32 = mybir.dt.float32
F32R = mybir.dt.float32r
AX = mybir.AxisListType
ALU = mybir.AluOpType
ACTF = mybir.ActivationFunctionType


def _r(ap):
    return ap.bitcast(F32R)


def _bcast_mid(ap2d, n):
    """(P, F) AP -> (P, n, F) AP with stride-0 middle dim."""
    return bass.AP(
        tensor=ap2d.tensor,
        offset=ap2d.offset,
        ap=[ap2d.ap[0], [0, n], ap2d.ap[-1]],
    )


def _rot_view(base_ap, col0, nseg):
    """View of base_ap cols [col0, col0+64*nseg) with halves swapped per
    64-wide segment: (P, nseg, 2, 32) reading [32:64] then [0:32]."""
    pstep = base_ap.ap[0]
    estep = base_ap.ap[-1][0]
    return bass.AP(
        tensor=base_ap.tensor,
        offset=base_ap.offset + (col0 + 32) * estep,
        ap=[pstep, [64 * estep, nseg], [-32 * estep, 2], [estep, 32]],
    )


def _seg_view(base_ap, col0, nseg):
    """(P, nseg, 64) view of base_ap cols [col0, col0+64*nseg)."""
    pstep = base_ap.ap[0]
    estep = base_ap.ap[-1][0]
    return bass.AP(
        tensor=base_ap.tensor,
        offset=base_ap.offset + col0 * estep,
        ap=[pstep, [64 * estep, nseg], [estep, 64]],
    )


def build_program():
    nc = bacc.Bacc(None, target_bir_lowering=False, debug=False)

    with tile.TileContext(nc) as tc:
        ctx = ExitStack()
        with ctx:
            dram = ctx.enter_context(tc.tile_pool(name="dram", bufs=1, space="DRAM"))
            xT_d = dram.tile([ND, 128, T], F32R, kind="ExternalInput", name="xT", uniquify=False)
            wqkv_d = dram.tile([ND, 128, 772], F32R, kind="ExternalInput", name="wqkv", uniquify=False)
            wo_d = dram.tile([G, HD, D], F32R, kind="ExternalInput", name="wo", uniquify=False)
            rqc_d = dram.tile([NT, 128, 64], F32, kind="ExternalInput", name="rqc", uniquify=False)
            rqs_d = dram.tile([NT, 128, 64], F32, kind="ExternalInput", name="rqs", uniquify=False)
            rkc_d = dram.tile([NT, 128, 64], F32, kind="ExternalInput", name="rkc", uniquify=False)
            rks_d = dram.tile([NT, 128, 64], F32, kind="ExternalInput", name="rks", uniquify=False)
            mtri_d = dram.tile([128, 128], F32, kind="ExternalInput", name="mtri", uniquify=False)
            ident_d = dram.tile([128, 128], F32, kind="ExternalInput", name="ident", uniquify=False)
            ones_d = dram.tile([1, 64], F32R, kind="ExternalInput", name="ones64", uniquify=False)
            vones_d = dram.tile([128, NT, G], F32R, kind="ExternalInput", name="vones", uniquify=False)
            outp_d = dram.tile([NT, 128, D], F32, kind="ExternalOutput", name="outp", uniquify=False)
            den_scr_d = dram.tile([G, T], F32R, name="den_scr")
            rec_scr_d = dram.tile([G, T], F32R, name="rec_scr")

            # ---- persistent SBUF (whole kernel) ----
            persist = ctx.enter_context(tc.tile_pool(name="persist", bufs=1))
            v_all = persist.tile([128, NT, 260], F32R)      # V' natural, 4x(64+ones)
            qt01 = persist.tile([128, T], F32R)             # Q^T heads 0,1 stacked
            qt23 = persist.tile([128, T], F32R)
            kt01 = persist.tile([128, T], F32R)
            kt23 = persist.tile([128, T], F32R)
            mtri_s = persist.tile([128, 128], F32)
            ident_s = persist.tile([128, 128], F32)
            ones_s = persist.tile([1, 64], F32R)
            rv_all = persist.tile([128, NT, 8], F32)       # rsqrt(mean+eps) per seg
            stats_all = persist.tile([128, NT, 8], F32)
            eps_s = persist.tile([128, 1], F32)
            nc.vector.memset(eps_s, EPS)

            nc.sync.dma_start(out=mtri_s, in_=mtri_d)
            nc.sync.dma_start(out=ident_s, in_=ident_d)
            nc.sync.dma_start(out=ones_s, in_=ones_d)

            # ================= Phase 1: projections + rmsnorm + rope + transpose
            p1 = ExitStack()
            with p1:
                xpool = p1.enter_context(tc.tile_pool(name="xpool", bufs=1))
                wpool = p1.enter_context(tc.tile_pool(name="wpool", bufs=1))
                tabpool = p1.enter_context(tc.tile_pool(name="tabpool", bufs=1))
                work1 = p1.enter_context(tc.tile_pool(name="work1", bufs=2))
                ps_qk = p1.enter_context(tc.tile_pool(name="ps_qk", bufs=2, space="PSUM"))
                ps_v = p1.enter_context(tc.tile_pool(name="ps_v", bufs=2, space="PSUM"))
                ps_tr = p1.enter_context(tc.tile_pool(name="ps_tr", bufs=1, space="PSUM"))

                w_s = wpool.tile([128, ND, 772], F32R)
                nc.sync.dma_start(out=w_s, in_=wqkv_d.rearrange("c p n -> p c n"))
                xt_s = []
                for c in range(ND):
                    xc = xpool.tile([128, T], F32R, name=f"xt{c}")
                    nc.sync.dma_start(out=xc, in_=xT_d[c])
                    xt_s.append(xc)
                rqc_s = tabpool.tile([128, NT, 64], F32)
                rqs_s = tabpool.tile([128, NT, 64], F32)
                rkc_s = tabpool.tile([128, NT, 64], F32)
                rks_s = tabpool.tile([128, NT, 64], F32)
                nc.sync.dma_start(out=rqc_s, in_=rqc_d.rearrange("t p n -> p t n"))
                nc.sync.dma_start(out=rqs_s, in_=rqs_d.rearrange("t p n -> p t n"))
                nc.sync.dma_start(out=rkc_s, in_=rkc_d.rearrange("t p n -> p t n"))
                nc.sync.dma_start(out=rks_s, in_=rks_d.rearrange("t p n -> p t n"))

                vones_cols = bass.AP(
                    tensor=v_all.tensor,
                    offset=v_all.offset + 64,
                    ap=[v_all.ap[0], [260, NT], [65, G]])
                nc.sync.dma_start(out=vones_cols, in_=vones_d)

                for r4 in range(NT // 4):  # rounds of 4 t-tiles
                    trp = [ps_tr.tile([128, 512], F32, name=f"tr{cb}", tag=f"tr{cb}")
                           for cb in range(4)]
                    for it in range(4 * r4, 4 * r4 + 4):
                        qkp = ps_qk.tile([128, 512], F32, tag="qk")
                        vp = ps_v.tile([128, 260], F32, tag="v")
                        for c in range(ND):
                            lhs = xt_s[c][:, it * 128:(it + 1) * 128]
                            nc.tensor.matmul(qkp, lhs, w_s[:, c, 0:512],
                                             start=(c == 0), stop=(c == ND - 1))
                            nc.tensor.matmul(vp, lhs, w_s[:, c, 512:772],
                                             start=(c == 0), stop=(c == ND - 1))
                        # drain PSUM, then rms stats from SBUF (PSUM has one
                        # DVE read port - can't read qkp twice in one op)
                        qraw = work1.tile([128, 512], F32, tag="qraw")
                        nc.vector.tensor_copy(qraw, qkp)
                        scr = work1.tile([128, 512], F32, tag="scr")
                        nc.vector.tensor_mul(out=scr, in0=qraw, in1=qraw)
                        nc.vector.tensor_reduce(
                            out=stats_all[:, it, :],
                            in_=scr.rearrange("p (s e) -> p s e", e=64),
                            axis=AX.X, op=ALU.add)
                        # rsqrt = exp(-0.5*ln(sumsq/64 + eps))
                        nc.scalar.activation(out=stats_all[:, it, :], in_=stats_all[:, it, :],
                                             func=ACTF.Ln, scale=1.0 / HD, bias=eps_s)
                        nc.scalar.activation(out=rv_all[:, it, :], in_=stats_all[:, it, :],
                                             func=ACTF.Exp, scale=-0.5)
                        # rms scale applied per segment
                        qhat = work1.tile([128, 512], F32, tag="qhat")
                        for s in range(8):
                            nc.vector.tensor_scalar_mul(
                                out=qhat[:, s * 64:(s + 1) * 64],
                                in0=qraw[:, s * 64:(s + 1) * 64],
                                scalar1=rv_all[:, it, s:s + 1])
                        # V drain (only the 4x64 value cols; ones cols DMA'd once)
                        vdst = bass.AP(
                            tensor=v_all.tensor,
                            offset=v_all[:, it, :].offset,
                            ap=[v_all.ap[0], [65, 4], [1, 64]])
                        vsrc = bass.AP(
                            tensor=vp.tensor,
                            offset=vp.offset,
                            ap=[vp.ap[0], [65, 4], [1, 64]])
                        nc.vector.tensor_copy(vdst, vsrc)
                        # rope (Q cols 0:256 w/ q tables, K cols 256:512 w/ k tables)
                        rot = work1.tile([128, 512], F32, tag="rot")
                        t1 = work1.tile([128, 512], F32, tag="t1")
                        for (c0, ctab, stab) in ((0, rqc_s, rqs_s), (256, rkc_s, rks_s)):
                            cb = _bcast_mid(ctab[:, it, :], 4)
                            sb = _bcast_mid(stab[:, it, :], 4)
                            sb4 = bass.AP(tensor=sb.tensor, offset=sb.offset,
                                          ap=[sb.ap[0], sb.ap[1], [32, 2], [1, 32]])
                            nc.vector.tensor_mul(
                                out=_seg_view(t1, c0, 4), in0=_seg_view(qhat, c0, 4), in1=cb)
                            nc.vector.tensor_tensor(
                                out=bass.AP(tensor=rot.tensor,
                                            offset=rot.offset + c0,
                                            ap=[rot.ap[0], [64, 4], [32, 2], [1, 32]]),
                                in0=_rot_view(qhat, c0, 4), in1=sb4, op=ALU.mult)
                            nc.vector.tensor_add(
                                out=_seg_view(t1, c0, 4), in0=_seg_view(t1, c0, 4),
                                in1=_seg_view(rot, c0, 4))
                        # transpose 4 col-blocks -> head-major (128, t) layouts
                        for cb in range(4):
                            nc.tensor.transpose(
                                trp[cb][:, (it % 4) * 128:(it % 4 + 1) * 128],
                                t1[:, cb * 128:(cb + 1) * 128], ident_s)
                    dsts = (qt01, qt23, kt01, kt23)
                    for cb in range(4):
                        nc.vector.tensor_copy(
                            dsts[cb][:, r4 * 512:(r4 + 1) * 512], trp[cb])

            # ================= Phases 2+3 share ot / wo_s
            p23 = ExitStack()
            with p23:
                otpool = p23.enter_context(tc.tile_pool(name="otpool", bufs=1))
                ot = [otpool.tile([65, T], F32R, name=f"ot{h}") for h in range(G)]
                wo_s = otpool.tile([HD, G, D], F32R)
                nc.sync.dma_start(out=wo_s, in_=wo_d.rearrange("h p n -> p h n"))

                # ---- Phase 2: attention per head
                p2 = p23.enter_context(ExitStack())
                ptpool = p2.enter_context(tc.tile_pool(name="ptpool", bufs=3))
                small = p2.enter_context(tc.tile_pool(name="small", bufs=2))
                ps_s = p2.enter_context(tc.tile_pool(name="ps_s", bufs=2, space="PSUM"))
                ps_o = p2.enter_context(tc.tile_pool(name="ps_o", bufs=2, space="PSUM"))

                for h in range(G):
                    qt = (qt01, qt01, qt23, qt23)[h]
                    kt = (kt01, kt01, kt23, kt23)[h]
                    pb = 64 * (h % 2)
                    for j in range(4):
                        kmax = 4 * (j + 1)
                        op = ps_o.tile([65, 512], F32, tag="ob")
                        for g0 in range(0, kmax, 3):
                            gn = min(3, kmax - g0)
                            sg = ps_s.tile([128, 3 * 512], F32, tag="sg")
                            pt = ptpool.tile([128, 3 * 512], F32R, tag="pt")
                            for i in range(g0, g0 + gn):
                                sl = slice((i - g0) * 512, (i - g0 + 1) * 512)
                                nc.tensor.matmul(
                                    sg[:, sl],
                                    kt[pb:pb + 64, i * 128:(i + 1) * 128],
                                    qt[pb:pb + 64, j * 512:(j + 1) * 512],
                                    start=True, stop=True)
                                r = i - 4 * j
                                if r >= 0:  # diagonal block mask
                                    dsl = slice((i - g0) * 512 + 128 * r,
                                                (i - g0) * 512 + 128 * (r + 1))
                                    nc.vector.tensor_add(
                                        out=sg[:, dsl], in0=sg[:, dsl], in1=mtri_s)
                            nc.scalar.activation(
                                out=pt[:, 0:gn * 512], in_=sg[:, 0:gn * 512],
                                func=ACTF.Exp)
                            for i in range(g0, g0 + gn):
                                r = i - 4 * j
                                c0 = max(0, 128 * r)
                                psl = slice((i - g0) * 512 + c0, (i - g0 + 1) * 512)
                                nc.tensor.matmul(
                                    op[:, c0:512],
                                    v_all[:, i, h * 65:(h + 1) * 65],
                                    pt[:, psl],
                                    start=(i == 0), stop=(i == kmax - 1))
                        nc.vector.tensor_copy(ot[h][:, j * 512:(j + 1) * 512], op)
                    # denominators -> reciprocal -> broadcast -> normalize
                    # (reshape (1,2048)<->(128,16) via DRAM scratch round-trip)
                    nc.sync.dma_start(out=den_scr_d[h], in_=ot[h][64:65, :])
                    den = small.tile([128, 16], F32R, tag="den")
                    nc.sync.dma_start(out=den, in_=den_scr_d[h].rearrange("(p f) -> p f", p=128))
                    rec = small.tile([128, 16], F32R, tag="rec")
                    with nc.allow_low_precision(reason="denominator recip in f32r"):
                        nc.vector.reciprocal(out=rec, in_=den)
                    nc.sync.dma_start(out=rec_scr_d[h], in_=rec)
                    rrow = small.tile([1, T], F32R, tag="rrow")
                    nc.sync.dma_start(out=rrow, in_=rec_scr_d[h].rearrange("(a f) -> a f", a=1))
                    for j in range(4):
                        bc = ps_o.tile([64, 512], F32, tag="ob")
                        nc.tensor.matmul(bc, ones_s, rrow[:, j * 512:(j + 1) * 512],
                                         start=True, stop=True)
                        nc.vector.tensor_mul(
                            out=ot[h][0:64, j * 512:(j + 1) * 512],
                            in0=ot[h][0:64, j * 512:(j + 1) * 512], in1=bc)
                p2.close()

                # ---- Phase 3: output projection
                outpool = p23.enter_context(tc.tile_pool(name="outpool", bufs=3))
                ps_op = p23.enter_context(tc.tile_pool(name="ps_op", bufs=4, space="PSUM"))
                for it in range(NT):
                    osb = outpool.tile([128, D], F32, tag="osb")
                    for n in range(2):
                        pso = ps_op.tile([128, 512], F32, tag="op")
                        for h in range(G):
                            nc.tensor.matmul(
                                pso,
                                ot[h][0:64, it * 128:(it + 1) * 128],
                                wo_s[:, h, n * 512:(n + 1) * 512],
                                start=(h == 0), stop=(h == G - 1))
                        nc.scalar.copy(osb[:, n * 512:(n + 1) * 512], pso)
                    nc.sync.dma_start(out=outp_d[it], in_=osb)

    nc.compile()
    return nc


_PROGRAM = None


def _get_program():
    global _PROGRAM
    if _PROGRAM is None:
        _PROGRAM = build_program()
    return _PROGRAM


def make_inputs_for_core(core, x, Wq, Wk, Wv, Wo, q_norm_w, k_norm_w):
    b, g = core // 4, core % 4
    xT = np.ascontiguousarray(x[b].T).reshape(ND, 128, T).astype(np.float32)
    wq = Wq[:, 256 * g:256 * (g + 1)]
    wk = Wk[:, 256 * g:256 * (g + 1)]
    wv = Wv[:, 256 * g:256 * (g + 1)]
    wvp = np.zeros((D, 260), np.float32)
    for h in range(G):
        wvp[:, h * 65:h * 65 + 64] = wv[:, h * 64:(h + 1) * 64]
    wqkv = np.concatenate([wq, wk, wvp], axis=1).reshape(ND, 128, 772)
    wqkv = np.ascontiguousarray(np.swapaxes(
        np.concatenate([wq, wk, wvp], axis=1).reshape(ND, 128, 772), 0, 0))
    wo = np.ascontiguousarray(
        Wo[256 * g:256 * (g + 1), :].reshape(G, HD, D)).astype(np.float32)

    inv_freq = 1.0 / (ROPE_BASE ** (np.arange(0, HD, 2, dtype=np.float64) / HD))
    tarr = np.arange(T, dtype=np.float64)
    fr = np.outer(tarr, inv_freq)
    cos, sin = np.cos(fr), np.sin(fr)

    def tables(w, scale):
        c = np.empty((T, HD), np.float64)
        s = np.empty((T, HD), np.float64)
        c[:, :32] = cos * w[:32] * scale
        c[:, 32:] = cos * w[32:] * scale
        s[:, :32] = -sin * w[32:] * scale
        s[:, 32:] = sin * w[:32] * scale
        return (c.astype(np.float32).reshape(NT, 128, 64),
                s.astype(np.float32).reshape(NT, 128, 64))

    qw = np.asarray(q_norm_w, np.float64)
    kw = np.asarray(k_norm_w, np.float64)
    rqc, rqs = tables(qw, 0.125)
    rkc, rks = tables(kw, 1.0)

    kp = np.arange(128)[:, None]
    qf = np.arange(128)[None, :]
    mtri = np.where(qf >= kp, 0.0, MASK_NEG).astype(np.float32)
    ident = np.eye(128, dtype=np.float32)
    ones64 = np.ones((1, 64), np.float32)

    vones = np.ones((128, NT, G), np.float32)
    return {
        "xT": xT.astype(np.float32), "wqkv": wqkv.astype(np.float32), "wo": wo,
        "rqc": rqc, "rqs": rqs, "rkc": rkc, "rks": rks,
        "mtri": mtri, "ident": ident, "ones64": ones64, "vones": vones,
    }


def run_on_hw(inputs, trace=False):
    from concourse.bass_utils import run_bass_kernel_spmd
    nc = _get_program()
    in_maps = [make_inputs_for_core(c, **inputs) for c in range(NCORES)]
    res = run_bass_kernel_spmd(nc, in_maps, list(range(NCORES)), trace=trace)
    parts = [res.results[c]["outp"].reshape(T, D) for c in range(NCORES)]
    out = np.stack([sum(parts[0:4]), sum(parts[4:8])]).astype(np.float32)
    return out, res


def kernel(**inputs):
    out, _ = run_on_hw(inputs, trace=False)
    return out



# revision 5
# speedup vs baseline: 1.0504x; 1.0504x over previous
"""MultiHeadAttention (QK-RMSNorm + RoPE + causal) Trainium2 Bass kernel.

Sharding: 8 cores = 2 batches x 4 head-groups (4 heads each).
Each core computes a partial (2048, 1024) output (its heads' contribution
through the output projection); host sums the 4 group-partials per batch.

v2: bf16 matmul inputs (fp32 PSUM accumulation), DMA-XBAR transposes,
single-table activations, fused rms apply, on-chip softmax denominators.
"""

import math
import sys
from contextlib import ExitStack

import numpy as np
import ml_dtypes

sys.path.insert(0, "/opt/trn_rl_repo")

import concourse.bass as bass  # noqa: E402
import concourse.bacc as bacc  # noqa: E402
import concourse.tile as tile  # noqa: E402
from concourse import mybir  # noqa: E402

B = 2
T = 2048
D = 1024
H = 16
HD = 64
G = 4  # heads per core
NCORES = 8
NT = T // 128  # 16 t-tiles
ND = D // 128  # 8 d-chunks
EPS = 1e-6
ROPE_BASE = 10000.0
MASK_NEG = -30000.0

F32 = mybir.dt.float32
BF16 = mybir.dt.bfloat16
AX = mybir.AxisListType
ALU = mybir.AluOpType
ACTF = mybir.ActivationFunctionType


def _rot_view(base_ap, col0, nseg):
    """View of base_ap cols [col0, col0+64*nseg) with halves swapped per
    64-wide segment: (P, nseg, 2, 32) reading [32:64] then [0:32]."""
    pstep = base_ap.ap[0]
    estep = base_ap.ap[-1][0]
    return bass.AP(
        tensor=base_ap.tensor,
        offset=base_ap.offset + (col0 + 32) * estep,
        ap=[pstep, [64 * estep, nseg], [-32 * estep, 2], [estep, 32]],
    )


def _seg_view(base_ap, col0, nseg):
    """(P, nseg, 64) view of base_ap cols [col0, col0+64*nseg)."""
    pstep = base_ap.ap[0]
    estep = base_ap.ap[-1][0]
    return bass.AP(
        tensor=base_ap.tensor,
        offset=base_ap.offset + col0 * estep,
        ap=[pstep, [64 * estep, nseg], [estep, 64]],
    )


def build_program():
    nc = bacc.Bacc(None, target_bir_lowering=False, debug=False)

    with tile.TileContext(nc) as tc:
        ctx = ExitStack()
        with ctx:
            dram = ctx.enter_context(tc.tile_pool(name="dram", bufs=1, space="DRAM"))
            xT_d = dram.tile([ND, 128, T], BF16, kind="ExternalInput", name="xT", uniquify=False)
            wqkv_d = dram.tile([ND, 128, 772], BF16, kind="ExternalInput", name="wqkv", uniquify=False)
            wo_d = dram.tile([G, HD, D], BF16, kind="ExternalInput", name="wo", uniquify=False)
            ctab_d = dram.tile([NT, 128, 128], BF16, kind="ExternalInput", name="ctab", uniquify=False)
            stab_d = dram.tile([NT, 128, 128], BF16, kind="ExternalInput", name="stab", uniquify=False)
            mtri_d = dram.tile([128, 128], F32, kind="ExternalInput", name="mtri", uniquify=False)
            vones_d = dram.tile([128, NT, G], BF16, kind="ExternalInput", name="vones", uniquify=False)
            outp_d = dram.tile([NT, 128, D], F32, kind="ExternalOutput", name="outp", uniquify=False)

            # ---- persistent SBUF (whole kernel) ----
            persist = ctx.enter_context(tc.tile_pool(name="persist", bufs=1))
            v_all = persist.tile([128, NT, 260], BF16)     # V' natural, 4x(64+ones)
            qt01 = persist.tile([128, T], BF16)            # Q^T heads 0,1 stacked
            qt23 = persist.tile([128, T], BF16)
            kt01 = persist.tile([128, T], BF16)
            kt23 = persist.tile([128, T], BF16)
            mtri_s = persist.tile([128, 128], F32)
            stats_all = persist.tile([128, NT, 8], F32)
            rv_all = persist.tile([128, NT, 8], F32)
            eps_s = persist.tile([128, 1], F32)
            nc.vector.memset(eps_s, EPS)

            nc.sync.dma_start(out=mtri_s, in_=mtri_d)

            # ================= Phase 1: projections + rmsnorm + rope + transpose
            p1 = ExitStack()
            with p1:
                xpool = p1.enter_context(tc.tile_pool(name="xpool", bufs=1))
                wpool = p1.enter_context(tc.tile_pool(name="wpool", bufs=1))
                tabpool = p1.enter_context(tc.tile_pool(name="tabpool", bufs=1))
                work1 = p1.enter_context(tc.tile_pool(name="work1", bufs=2))
                ps_qk = p1.enter_context(tc.tile_pool(name="ps_qk", bufs=2, space="PSUM"))
                ps_v = p1.enter_context(tc.tile_pool(name="ps_v", bufs=2, space="PSUM"))

                w_s = wpool.tile([128, ND, 772], BF16)
                nc.sync.dma_start(out=w_s, in_=wqkv_d.rearrange("c p n -> p c n"))
                xt_s = []
                for c in range(ND):
                    xc = xpool.tile([128, T], BF16, name=f"xt{c}")
                    nc.sync.dma_start(out=xc, in_=xT_d[c])
                    xt_s.append(xc)
                ctab_s = tabpool.tile([128, NT, 128], BF16)
                stab_s = tabpool.tile([128, NT, 128], BF16)
                nc.sync.dma_start(out=ctab_s, in_=ctab_d.rearrange("t p n -> p t n"))
                nc.sync.dma_start(out=stab_s, in_=stab_d.rearrange("t p n -> p t n"))

                vones_cols = bass.AP(
                    tensor=v_all.tensor,
                    offset=v_all.offset + 64,
                    ap=[v_all.ap[0], [260, NT], [65, G]])
                nc.sync.dma_start(out=vones_cols, in_=vones_d)

                dsts = (qt01, qt23, kt01, kt23)
                for it in range(NT):
                    qkp = ps_qk.tile([128, 512], F32, tag="qk")
                    vp = ps_v.tile([128, 260], F32, tag="v")
                    for c in range(ND):
                        lhs = xt_s[c][:, it * 128:(it + 1) * 128]
                        nc.tensor.matmul(qkp, lhs, w_s[:, c, 0:512],
                                         start=(c == 0), stop=(c == ND - 1))
                        nc.tensor.matmul(vp, lhs, w_s[:, c, 512:772],
                                         start=(c == 0), stop=(c == ND - 1))
                    # rms stats: squares on ACT (PSUM read 1), seg-sums on DVE
                    scr = work1.tile([128, 512], BF16, tag="scr")
                    nc.scalar.activation(out=scr, in_=qkp, func=ACTF.Square)
                    nc.vector.tensor_reduce(
                        out=stats_all[:, it, :],
                        in_=scr.rearrange("p (s e) -> p s e", e=64),
                        axis=AX.X, op=ALU.add)
                    # rv = 1/sqrt(sumsq/64 + eps): ACT sqrt + fast DVE recip
                    nc.scalar.activation(out=stats_all[:, it, :], in_=stats_all[:, it, :],
                                         func=ACTF.Sqrt,
                                         scale=1.0 / HD, bias=eps_s)
                    with nc.allow_low_precision(reason="rms scale recip"):
                        nc.vector.reciprocal_approx_fast(
                            out=rv_all[:, it, :], in_=stats_all[:, it, :])
                    # fused rms apply: qhat = qkp * rv (per 64-seg), PSUM read 2
                    qhat = work1.tile([128, 512], BF16, tag="qhat")
                    rvb = bass.AP(
                        tensor=rv_all.tensor,
                        offset=rv_all[:, it, :].offset,
                        ap=[rv_all.ap[0], [1, 8], [0, 64]])
                    nc.vector.tensor_tensor(
                        out=qhat.rearrange("p (s e) -> p s e", e=64),
                        in0=qkp.rearrange("p (s e) -> p s e", e=64),
                        in1=rvb, op=ALU.mult)
                    # V drain (only the 4x64 value cols; ones cols DMA'd once)
                    vdst = bass.AP(
                        tensor=v_all.tensor,
                        offset=v_all[:, it, :].offset,
                        ap=[v_all.ap[0], [65, 4], [1, 64]])
                    vsrc = bass.AP(
                        tensor=vp.tensor,
                        offset=vp.offset,
                        ap=[vp.ap[0], [65, 4], [1, 64]])
                    nc.vector.tensor_copy(vdst, vsrc)
                    # rope: t1 = qhat*c + rot(qhat)*s  (tables have q|k halves)
                    t1 = work1.tile([128, 512], BF16, tag="t1")
                    rot = work1.tile([128, 512], BF16, tag="rot")
                    cb_view = bass.AP(
                        tensor=ctab_s.tensor,
                        offset=ctab_s[:, it, :].offset,
                        ap=[ctab_s.ap[0], [64, 2], [0, 4], [1, 64]])
                    nc.vector.tensor_tensor(
                        out=t1.rearrange("p (h r e) -> p h r e", h=2, r=4),
                        in0=qhat.rearrange("p (h r e) -> p h r e", h=2, r=4),
                        in1=cb_view, op=ALU.mult)
                    for half, c0 in ((0, 0), (1, 256)):
                        sb_view = bass.AP(
                            tensor=stab_s.tensor,
                            offset=stab_s[:, it, :].offset + 64 * half,
                            ap=[stab_s.ap[0], [0, 4], [32, 2], [1, 32]])
                        rot_out = bass.AP(
                            tensor=rot.tensor,
                            offset=rot.offset + c0,
                            ap=[rot.ap[0], [64, 4], [32, 2], [1, 32]])
                        nc.vector.tensor_tensor(
                            out=rot_out, in0=_rot_view(qhat, c0, 4),
                            in1=sb_view, op=ALU.mult)
                    nc.vector.tensor_add(out=t1, in0=t1, in1=rot)
                    # transpose 4 col-blocks -> head-major (128, t) via DMA XBAR
                    for cb in range(4):
                        nc.sync.dma_start_transpose(
                            out=dsts[cb][:, it * 128:(it + 1) * 128],
                            in_=t1[:, cb * 128:(cb + 1) * 128])

            # ================= Phases 2+3 share ot / wo_s
            p23 = ExitStack()
            with p23:
                otpool = p23.enter_context(tc.tile_pool(name="otpool", bufs=1))
                ot = [otpool.tile([64, T], BF16, name=f"ot{h}") for h in range(G)]
                wo_s = otpool.tile([HD, G, D], BF16)
                nc.sync.dma_start(out=wo_s, in_=wo_d.rearrange("h p n -> p h n"))

                # ---- Phase 2: attention per head
                p2 = p23.enter_context(ExitStack())
                ptpool = p2.enter_context(tc.tile_pool(name="ptpool", bufs=3))
                npool = p2.enter_context(tc.tile_pool(name="npool", bufs=2))
                ps_s = p2.enter_context(tc.tile_pool(name="ps_s", bufs=2, space="PSUM"))
                ps_o = p2.enter_context(tc.tile_pool(name="ps_o", bufs=2, space="PSUM"))

                for h in range(G):
                    qt = (qt01, qt01, qt23, qt23)[h]
                    kt = (kt01, kt01, kt23, kt23)[h]
                    pb = 64 * (h % 2)
                    for j in range(4):
                        kmax = 4 * (j + 1)
                        op = ps_o.tile([65, 512], F32, tag="ob")
                        for g0 in range(0, kmax, 3):
                            gn = min(3, kmax - g0)
                            sg = ps_s.tile([128, 3 * 512], F32, tag="sg")
                            pt = ptpool.tile([128, 3 * 512], BF16, tag="pt")
                            for i in range(g0, g0 + gn):
                                sl = slice((i - g0) * 512, (i - g0 + 1) * 512)
                                nc.tensor.matmul(
                                    sg[:, sl],
                                    kt[pb:pb + 64, i * 128:(i + 1) * 128],
                                    qt[pb:pb + 64, j * 512:(j + 1) * 512],
                                    start=True, stop=True)
                                r = i - 4 * j
                                if r >= 0:  # diagonal block mask
                                    dsl = slice((i - g0) * 512 + 128 * r,
                                                (i - g0) * 512 + 128 * (r + 1))
                                    nc.vector.tensor_add(
                                        out=sg[:, dsl], in0=sg[:, dsl], in1=mtri_s)
                            nc.scalar.activation(
                                out=pt[:, 0:gn * 512], in_=sg[:, 0:gn * 512],
                                func=ACTF.Exp)
                            for i in range(g0, g0 + gn):
                                r = i - 4 * j
                                c0 = max(0, 128 * r)
                                psl = slice((i - g0) * 512 + c0, (i - g0 + 1) * 512)
                                nc.tensor.matmul(
                                    op[:, c0:512],
                                    v_all[:, i, h * 65:(h + 1) * 65],
                                    pt[:, psl],
                                    start=(i == 0), stop=(i == kmax - 1))
                        # denominator -> reciprocal -> DMA broadcast -> normalize
                        den_sb = npool.tile([1, 512], F32, tag="den")
                        nc.vector.tensor_copy(den_sb, op[64:65, 0:512])
                        rec_sb = npool.tile([1, 512], F32, tag="rec")
                        with nc.allow_low_precision(reason="softmax denominator recip"):
                            nc.vector.reciprocal_approx_fast(
                                out=rec_sb, in_=den_sb)
                        rec64 = npool.tile([64, 512], F32, tag="rec64")
                        rec_b = bass.AP(
                            tensor=rec_sb.tensor,
                            offset=rec_sb.offset,
                            ap=[[1, 1], [0, 64], rec_sb.ap[-1]])
                        nc.sync.dma_start(out=rec64, in_=rec_b)
                        nc.vector.tensor_mul(
                            out=ot[h][:, j * 512:(j + 1) * 512],
                            in0=op[0:64, 0:512], in1=rec64)
                p2.close()

                # ---- Phase 3: output projection
                outpool = p23.enter_context(tc.tile_pool(name="outpool", bufs=3))
                ps_op = p23.enter_context(tc.tile_pool(name="ps_op", bufs=4, space="PSUM"))
                for it in range(NT):
                    osb = outpool.tile([128, D], F32, tag="osb")
                    for n in range(2):
                        pso = ps_op.tile([128, 512], F32, tag="op")
                        for h in range(G):
                            nc.tensor.matmul(
                                pso,
                                ot[h][:, it * 128:(it + 1) * 128],
                                wo_s[:, h, n * 512:(n + 1) * 512],
                                start=(h == 0), stop=(h == G - 1))
                        nc.scalar.copy(osb[:, n * 512:(n + 1) * 512], pso)
                    nc.sync.dma_start(out=outp_d[it], in_=osb)

    nc.compile()
    return nc


_PROGRAM = None


def _get_program():
    global _PROGRAM
    if _PROGRAM is None:
        _PROGRAM = build_program()
    return _PROGRAM


def make_inputs_for_core(core, x, Wq, Wk, Wv, Wo, q_norm_w, k_norm_w):
    bf16 = ml_dtypes.bfloat16
    b, g = core // 4, core % 4
    xT = np.ascontiguousarray(np.asarray(x[b]).T).reshape(ND, 128, T)
    wq = np.asarray(Wq)[:, 256 * g:256 * (g + 1)]
    wk = np.asarray(Wk)[:, 256 * g:256 * (g + 1)]
    wv = np.asarray(Wv)[:, 256 * g:256 * (g + 1)]
    wvp = np.zeros((D, 260), np.float32)
    for h in range(G):
        wvp[:, h * 65:h * 65 + 64] = wv[:, h * 64:(h + 1) * 64]
    wqkv = np.concatenate([wq, wk, wvp], axis=1).reshape(ND, 128, 772)
    wo = np.ascontiguousarray(
        np.asarray(Wo)[256 * g:256 * (g + 1), :].reshape(G, HD, D))

    inv_freq = 1.0 / (ROPE_BASE ** (np.arange(0, HD, 2, dtype=np.float64) / HD))
    tarr = np.arange(T, dtype=np.float64)
    fr = np.outer(tarr, inv_freq)
    cos, sin = np.cos(fr), np.sin(fr)

    def tables(w, scale):
        c = np.empty((T, HD), np.float64)
        s = np.empty((T, HD), np.float64)
        c[:, :32] = cos * w[:32] * scale
        c[:, 32:] = cos * w[32:] * scale
        s[:, :32] = -sin * w[32:] * scale
        s[:, 32:] = sin * w[:32] * scale
        return c, s

    qw = np.asarray(q_norm_w, np.float64)
    kw = np.asarray(k_norm_w, np.float64)
    cq, sq = tables(qw, 0.125)
    ck, sk = tables(kw, 1.0)
    ctab = np.concatenate([cq, ck], axis=1).reshape(NT, 128, 128)
    stab = np.concatenate([sq, sk], axis=1).reshape(NT, 128, 128)

    kp = np.arange(128)[:, None]
    qf = np.arange(128)[None, :]
    mtri = np.where(qf >= kp, 0.0, MASK_NEG).astype(np.float32)
    vones = np.ones((128, NT, G), bf16)
    return {
        "xT": xT.astype(bf16), "wqkv": wqkv.astype(bf16), "wo": wo.astype(bf16),
        "ctab": ctab.astype(bf16), "stab": stab.astype(bf16),
        "mtri": mtri, "vones": vones,
    }


def run_on_hw(inputs, trace=False):
    from concourse.bass_utils import run_bass_kernel_spmd
    nc = _get_program()
    in_maps = [make_inputs_for_core(c, **inputs) for c in range(NCORES)]
    res = run_bass_kernel_spmd(nc, in_maps, list(range(NCORES)), trace=trace)
    parts = [res.results[c]["outp"].reshape(T, D) for c in range(NCORES)]
    out = np.stack([sum(parts[0:4]), sum(parts[4:8])]).astype(np.float32)
    return out, res


def kernel(**inputs):
    out, _ = run_on_hw(inputs, trace=False)
    return out


# revision 9
# speedup vs baseline: 1.2516x; 1.1916x over previous
"""MultiHeadAttention (QK-RMSNorm + RoPE + causal) Trainium2 Bass kernel.

Sharding: 8 cores = 2 batches x 4 head-groups (4 heads each).
Each core computes a partial (2048, 1024) output (its heads' contribution
through the output projection); host sums the 4 group-partials per batch.

v2: bf16 matmul inputs (fp32 PSUM accumulation), DMA-XBAR transposes,
single-table activations, fused rms apply, on-chip softmax denominators.
"""

import math
import sys
from contextlib import ExitStack

import numpy as np
import ml_dtypes

sys.path.insert(0, "/opt/trn_rl_repo")

import concourse.bass as bass  # noqa: E402
import concourse.bacc as bacc  # noqa: E402
import concourse.tile as tile  # noqa: E402
from concourse import mybir  # noqa: E402

B = 2
T = 2048
D = 1024
H = 16
HD = 64
G = 4  # heads per core
NCORES = 8
NT = T // 128  # 16 t-tiles
ND = D // 128  # 8 d-chunks
EPS = 1e-6
ROPE_BASE = 10000.0
MASK_NEG = -30000.0

F32 = mybir.dt.float32
BF16 = mybir.dt.bfloat16
AX = mybir.AxisListType
ALU = mybir.AluOpType
ACTF = mybir.ActivationFunctionType


def _rot_view(base_ap, col0, nseg):
    """View of base_ap cols [col0, col0+64*nseg) with halves swapped per
    64-wide segment: (P, nseg, 2, 32) reading [32:64] then [0:32]."""
    pstep = base_ap.ap[0]
    estep = base_ap.ap[-1][0]
    return bass.AP(
        tensor=base_ap.tensor,
        offset=base_ap.offset + (col0 + 32) * estep,
        ap=[pstep, [64 * estep, nseg], [-32 * estep, 2], [estep, 32]],
    )


def _seg_view(base_ap, col0, nseg):
    """(P, nseg, 64) view of base_ap cols [col0, col0+64*nseg)."""
    pstep = base_ap.ap[0]
    estep = base_ap.ap[-1][0]
    return bass.AP(
        tensor=base_ap.tensor,
        offset=base_ap.offset + col0 * estep,
        ap=[pstep, [64 * estep, nseg], [estep, 64]],
    )


def build_program():
    nc = bacc.Bacc(None, target_bir_lowering=False, debug=False)

    with tile.TileContext(nc) as tc:
        ctx = ExitStack()
        with ctx:
            dram = ctx.enter_context(tc.tile_pool(name="dram", bufs=1, space="DRAM"))
            xT_d = dram.tile([ND, 128, T], BF16, kind="ExternalInput", name="xT", uniquify=False)
            wqkv_d = dram.tile([ND, 128, 772], BF16, kind="ExternalInput", name="wqkv", uniquify=False)
            wo_d = dram.tile([2, 128, D], BF16, kind="ExternalInput", name="wo", uniquify=False)
            ctab_d = dram.tile([NT, 128, 128], BF16, kind="ExternalInput", name="ctab", uniquify=False)
            stab_d = dram.tile([NT, 128, 128], BF16, kind="ExternalInput", name="stab", uniquify=False)
            mtri_d = dram.tile([128, 128], F32, kind="ExternalInput", name="mtri", uniquify=False)
            vones_d = dram.tile([128, NT, G], BF16, kind="ExternalInput", name="vones", uniquify=False)
            outp_d = dram.tile([NT, 128, D], F32, kind="ExternalOutput", name="outp", uniquify=False)

            # ---- persistent SBUF (whole kernel) ----
            persist = ctx.enter_context(tc.tile_pool(name="persist", bufs=1))
            v_all = persist.tile([128, NT, 260], BF16)     # V' natural, 4x(64+ones)
            qt01 = persist.tile([128, T], BF16)            # Q^T heads 0,1 stacked
            qt23 = persist.tile([128, T], BF16)
            kt01 = persist.tile([128, T], BF16)
            kt23 = persist.tile([128, T], BF16)
            mtri_s = persist.tile([128, 128], F32)
            stats_all = persist.tile([128, NT, 8], F32)
            rv_all = persist.tile([128, NT, 8], F32)
            eps_s = persist.tile([128, 1], F32)
            nc.vector.memset(eps_s, EPS)

            nc.sync.dma_start(out=mtri_s, in_=mtri_d)

            # ================= Phase 1: projections + rmsnorm + rope + transpose
            p1 = ExitStack()
            with p1:
                xpool = p1.enter_context(tc.tile_pool(name="xpool", bufs=1))
                wpool = p1.enter_context(tc.tile_pool(name="wpool", bufs=1))
                tabpool = p1.enter_context(tc.tile_pool(name="tabpool", bufs=1))
                work1 = p1.enter_context(tc.tile_pool(name="work1", bufs=2))
                ps_qk = p1.enter_context(tc.tile_pool(name="ps_qk", bufs=2, space="PSUM"))
                ps_v = p1.enter_context(tc.tile_pool(name="ps_v", bufs=2, space="PSUM"))

                w_s = wpool.tile([128, ND, 772], BF16)
                nc.sync.dma_start(out=w_s, in_=wqkv_d.rearrange("c p n -> p c n"))
                xt_s = []
                for c in range(ND):
                    xc = xpool.tile([128, T], BF16, name=f"xt{c}")
                    nc.sync.dma_start(out=xc, in_=xT_d[c])
                    xt_s.append(xc)
                ctab_s = tabpool.tile([128, NT, 128], BF16)
                stab_s = tabpool.tile([128, NT, 128], BF16)
                nc.sync.dma_start(out=ctab_s, in_=ctab_d.rearrange("t p n -> p t n"))
                nc.sync.dma_start(out=stab_s, in_=stab_d.rearrange("t p n -> p t n"))

                vones_cols = bass.AP(
                    tensor=v_all.tensor,
                    offset=v_all.offset + 64,
                    ap=[v_all.ap[0], [260, NT], [65, G]])
                nc.sync.dma_start(out=vones_cols, in_=vones_d)

                dsts = (qt01, qt23, kt01, kt23)
                for it in range(NT):
                    qkp = ps_qk.tile([128, 512], F32, tag="qk")
                    vp = ps_v.tile([128, 260], F32, tag="v")
                    for c in range(ND):
                        lhs = xt_s[c][:, it * 128:(it + 1) * 128]
                        nc.tensor.matmul(qkp, lhs, w_s[:, c, 0:512],
                                         start=(c == 0), stop=(c == ND - 1))
                        nc.tensor.matmul(vp, lhs, w_s[:, c, 512:772],
                                         start=(c == 0), stop=(c == ND - 1))
                    # rms stats: squares on ACT (PSUM read 1), seg-sums on DVE
                    scr = work1.tile([128, 512], BF16, tag="scr")
                    nc.scalar.activation(out=scr, in_=qkp, func=ACTF.Square)
                    nc.vector.tensor_reduce(
                        out=stats_all[:, it, :],
                        in_=scr.rearrange("p (s e) -> p s e", e=64),
                        axis=AX.X, op=ALU.add)
                    # rv = 1/sqrt(sumsq/64 + eps): ACT sqrt + fast DVE recip
                    nc.scalar.activation(out=stats_all[:, it, :], in_=stats_all[:, it, :],
                                         func=ACTF.Sqrt,
                                         scale=1.0 / HD, bias=eps_s)
                    with nc.allow_low_precision(reason="rms scale recip"):
                        nc.vector.reciprocal_approx_fast(
                            out=rv_all[:, it, :], in_=stats_all[:, it, :])
                    # fused rms apply: qhat = qkp * rv (per 64-seg), PSUM read 2
                    qhat = work1.tile([128, 512], BF16, tag="qhat")
                    rvb = bass.AP(
                        tensor=rv_all.tensor,
                        offset=rv_all[:, it, :].offset,
                        ap=[rv_all.ap[0], [1, 8], [0, 64]])
                    nc.vector.tensor_tensor(
                        out=qhat.rearrange("p (s e) -> p s e", e=64),
                        in0=qkp.rearrange("p (s e) -> p s e", e=64),
                        in1=rvb, op=ALU.mult)
                    # V drain (only the 4x64 value cols; ones cols DMA'd once)
                    vdst = bass.AP(
                        tensor=v_all.tensor,
                        offset=v_all[:, it, :].offset,
                        ap=[v_all.ap[0], [65, 4], [1, 64]])
                    vsrc = bass.AP(
                        tensor=vp.tensor,
                        offset=vp.offset,
                        ap=[vp.ap[0], [65, 4], [1, 64]])
                    nc.vector.tensor_copy(vdst, vsrc)
                    # rope: t1 = qhat*c + rot(qhat)*s  (tables have q|k halves)
                    t1 = work1.tile([128, 512], BF16, tag="t1")
                    rot = work1.tile([128, 512], BF16, tag="rot")
                    cb_view = bass.AP(
                        tensor=ctab_s.tensor,
                        offset=ctab_s[:, it, :].offset,
                        ap=[ctab_s.ap[0], [64, 2], [0, 4], [1, 64]])
                    nc.vector.tensor_tensor(
                        out=t1.rearrange("p (h r e) -> p h r e", h=2, r=4),
                        in0=qhat.rearrange("p (h r e) -> p h r e", h=2, r=4),
                        in1=cb_view, op=ALU.mult)
                    for half, c0 in ((0, 0), (1, 256)):
                        sb_view = bass.AP(
                            tensor=stab_s.tensor,
                            offset=stab_s[:, it, :].offset + 64 * half,
                            ap=[stab_s.ap[0], [0, 4], [32, 2], [1, 32]])
                        rot_out = bass.AP(
                            tensor=rot.tensor,
                            offset=rot.offset + c0,
                            ap=[rot.ap[0], [64, 4], [32, 2], [1, 32]])
                        nc.vector.tensor_tensor(
                            out=rot_out, in0=_rot_view(qhat, c0, 4),
                            in1=sb_view, op=ALU.mult)
                    nc.vector.tensor_add(out=t1, in0=t1, in1=rot)
                    # transpose 4 col-blocks -> head-major (128, t) via DMA XBAR
                    for cb in range(4):
                        nc.sync.dma_start_transpose(
                            out=dsts[cb][:, it * 128:(it + 1) * 128],
                            in_=t1[:, cb * 128:(cb + 1) * 128])

            # ====== Phases 2+3 fused: pair-packed attention + oproj bursts
            p23 = ExitStack()
            with p23:
                otpool = p23.enter_context(tc.tile_pool(name="otpool", bufs=1))
                # heads stacked per pair: rows 0-63 = even head, 64-127 = odd
                ot01 = otpool.tile([128, T], BF16)
                ot23 = otpool.tile([128, T], BF16)
                wo_s = otpool.tile([128, 2, D], BF16)  # [hv-pair rows, pair, D]
                nc.sync.dma_start(out=wo_s, in_=wo_d.rearrange("a p n -> p a n"))

                ptpool = p23.enter_context(tc.tile_pool(name="ptpool", bufs=3))
                npool = p23.enter_context(tc.tile_pool(name="npool", bufs=2))
                outpool = p23.enter_context(tc.tile_pool(name="outpool", bufs=3))
                ps_sg = p23.enter_context(tc.tile_pool(name="ps_sg", bufs=3, space="PSUM"))
                ps_o = p23.enter_context(tc.tile_pool(name="ps_o", bufs=1, space="PSUM"))

                for j in range(4):
                    kmax = 4 * (j + 1)
                    for pair in range(2):
                        qt = (qt01, qt23)[pair]
                        kt = (kt01, kt23)[pair]
                        ott = (ot01, ot23)[pair]
                        opA = ps_o.tile([65, 512], F32, tag="opA")
                        opB = ps_o.tile([65, 512], F32, tag="opB")
                        for i in range(kmax):
                            sg = ps_sg.tile([128, 1024], F32, tag="sg")
                            pt = ptpool.tile([128, 1024], BF16, tag="pt")
                            nc.tensor.matmul(
                                sg[:, 0:512],
                                kt[0:64, i * 128:(i + 1) * 128],
                                qt[0:64, j * 512:(j + 1) * 512],
                                start=True, stop=True)
                            nc.tensor.matmul(
                                sg[:, 512:1024],
                                kt[64:128, i * 128:(i + 1) * 128],
                                qt[64:128, j * 512:(j + 1) * 512],
                                start=True, stop=True)
                            r = i - 4 * j
                            c0 = max(0, 128 * r) if r >= 0 else 0
                            if r >= 0:  # diagonal block mask, both heads
                                for half in range(2):
                                    dsl = slice(half * 512 + 128 * r,
                                                half * 512 + 128 * (r + 1))
                                    nc.vector.tensor_add(
                                        out=sg[:, dsl], in0=sg[:, dsl], in1=mtri_s)
                            nc.scalar.activation(out=pt, in_=sg, func=ACTF.Exp)
                            nc.tensor.matmul(
                                opA[:, c0:512],
                                v_all[:, i, (2 * pair) * 65:(2 * pair + 1) * 65],
                                pt[:, c0:512],
                                start=(i == 0), stop=(i == kmax - 1))
                            nc.tensor.matmul(
                                opB[:, c0:512],
                                v_all[:, i, (2 * pair + 1) * 65:(2 * pair + 2) * 65],
                                pt[:, 512 + c0:1024],
                                start=(i == 0), stop=(i == kmax - 1))
                        # denominators -> recip -> DMA broadcast -> normalize
                        for half, op in ((0, opA), (1, opB)):
                            den_sb = npool.tile([1, 512], F32, tag=f"den{half}")
                            nc.vector.tensor_copy(den_sb, op[64:65, 0:512])
                            rec_sb = npool.tile([1, 512], F32, tag=f"rec{half}")
                            with nc.allow_low_precision(reason="softmax den recip"):
                                nc.vector.reciprocal_approx_fast(
                                    out=rec_sb, in_=den_sb)
                            rec64 = npool.tile([64, 512], F32, tag=f"rec64{half}")
                            rec_b = bass.AP(
                                tensor=rec_sb.tensor,
                                offset=rec_sb.offset,
                                ap=[[1, 1], [0, 64], rec_sb.ap[-1]])
                            nc.sync.dma_start(out=rec64, in_=rec_b)
                            nc.vector.tensor_mul(
                                out=ott[64 * half:64 * (half + 1),
                                        j * 512:(j + 1) * 512],
                                in0=op[0:64, 0:512], in1=rec64)
                    # ---- oproj burst for t-tiles of this j-block
                    for it in range(4 * j, 4 * j + 4):
                        pso = ps_sg.tile([128, 1024], F32, tag="sg")
                        for n in range(2):
                            psl = slice(n * 512, (n + 1) * 512)
                            nc.tensor.matmul(
                                pso[:, psl],
                                ot01[:, it * 128:(it + 1) * 128],
                                wo_s[:, 0, n * 512:(n + 1) * 512],
                                start=True, stop=False)
                            nc.tensor.matmul(
                                pso[:, psl],
                                ot23[:, it * 128:(it + 1) * 128],
                                wo_s[:, 1, n * 512:(n + 1) * 512],
                                start=False, stop=True)
                        osb = outpool.tile([128, D], F32, tag="osb")
                        nc.scalar.copy(osb, pso)
                        nc.sync.dma_start(out=outp_d[it], in_=osb)

    nc.compile()
    return nc


_PROGRAM = None


def _get_program():
    global _PROGRAM
    if _PROGRAM is None:
        _PROGRAM = build_program()
    return _PROGRAM


def make_inputs_for_core(core, x, Wq, Wk, Wv, Wo, q_norm_w, k_norm_w):
    bf16 = ml_dtypes.bfloat16
    b, g = core // 4, core % 4
    xT = np.ascontiguousarray(np.asarray(x[b]).T).reshape(ND, 128, T)
    wq = np.asarray(Wq)[:, 256 * g:256 * (g + 1)]
    wk = np.asarray(Wk)[:, 256 * g:256 * (g + 1)]
    wv = np.asarray(Wv)[:, 256 * g:256 * (g + 1)]
    wvp = np.zeros((D, 260), np.float32)
    for h in range(G):
        wvp[:, h * 65:h * 65 + 64] = wv[:, h * 64:(h + 1) * 64]
    wqkv = np.concatenate([wq, wk, wvp], axis=1).reshape(ND, 128, 772)
    wo = np.ascontiguousarray(
        np.asarray(Wo)[256 * g:256 * (g + 1), :].reshape(2, 128, D))

    inv_freq = 1.0 / (ROPE_BASE ** (np.arange(0, HD, 2, dtype=np.float64) / HD))
    tarr = np.arange(T, dtype=np.float64)
    fr = np.outer(tarr, inv_freq)
    cos, sin = np.cos(fr), np.sin(fr)

    def tables(w, scale):
        c = np.empty((T, HD), np.float64)
        s = np.empty((T, HD), np.float64)
        c[:, :32] = cos * w[:32] * scale
        c[:, 32:] = cos * w[32:] * scale
        s[:, :32] = -sin * w[32:] * scale
        s[:, 32:] = sin * w[:32] * scale
        return c, s

    qw = np.asarray(q_norm_w, np.float64)
    kw = np.asarray(k_norm_w, np.float64)
    cq, sq = tables(qw, 0.125)
    ck, sk = tables(kw, 1.0)
    ctab = np.concatenate([cq, ck], axis=1).reshape(NT, 128, 128)
    stab = np.concatenate([sq, sk], axis=1).reshape(NT, 128, 128)

    kp = np.arange(128)[:, None]
    qf = np.arange(128)[None, :]
    mtri = np.where(qf >= kp, 0.0, MASK_NEG).astype(np.float32)
    vones = np.ones((128, NT, G), bf16)
    return {
        "xT": xT.astype(bf16), "wqkv": wqkv.astype(bf16), "wo": wo.astype(bf16),
        "ctab": ctab.astype(bf16), "stab": stab.astype(bf16),
        "mtri": mtri, "vones": vones,
    }


def run_on_hw(inputs, trace=False):
    from concourse.bass_utils import run_bass_kernel_spmd
    nc = _get_program()
    in_maps = [make_inputs_for_core(c, **inputs) for c in range(NCORES)]
    res = run_bass_kernel_spmd(nc, in_maps, list(range(NCORES)), trace=trace)
    parts = [res.results[c]["outp"].reshape(T, D) for c in range(NCORES)]
    out = np.stack([sum(parts[0:4]), sum(parts[4:8])]).astype(np.float32)
    return out, res


def kernel(**inputs):
    out, _ = run_on_hw(inputs, trace=False)
    return out


# revision 11
# speedup vs baseline: 1.3497x; 1.0784x over previous
"""MultiHeadAttention (QK-RMSNorm + RoPE + causal) Trainium2 Bass kernel.

Sharding: 8 cores = 2 batches x 4 head-groups (4 heads each).
Each core computes a partial (2048, 1024) output (its heads' contribution
through the output projection); host sums the 4 group-partials per batch.

v2: bf16 matmul inputs (fp32 PSUM accumulation), DMA-XBAR transposes,
single-table activations, fused rms apply, on-chip softmax denominators.
"""

import math
import sys
from contextlib import ExitStack

import numpy as np
import ml_dtypes

sys.path.insert(0, "/opt/trn_rl_repo")

import concourse.bass as bass  # noqa: E402
import concourse.bacc as bacc  # noqa: E402
import concourse.tile as tile  # noqa: E402
from concourse import mybir  # noqa: E402

B = 2
T = 2048
D = 1024
H = 16
HD = 64
G = 4  # heads per core
NCORES = 8
NT = T // 128  # 16 t-tiles
ND = D // 128  # 8 d-chunks
EPS = 1e-6
ROPE_BASE = 10000.0
MASK_NEG = -30000.0

F32 = mybir.dt.float32
BF16 = mybir.dt.bfloat16
AX = mybir.AxisListType
ALU = mybir.AluOpType
ACTF = mybir.ActivationFunctionType


def _rot_view(base_ap, col0, nseg):
    """View of base_ap cols [col0, col0+64*nseg) with halves swapped per
    64-wide segment: (P, nseg, 2, 32) reading [32:64] then [0:32]."""
    pstep = base_ap.ap[0]
    estep = base_ap.ap[-1][0]
    return bass.AP(
        tensor=base_ap.tensor,
        offset=base_ap.offset + (col0 + 32) * estep,
        ap=[pstep, [64 * estep, nseg], [-32 * estep, 2], [estep, 32]],
    )


def _seg_view(base_ap, col0, nseg):
    """(P, nseg, 64) view of base_ap cols [col0, col0+64*nseg)."""
    pstep = base_ap.ap[0]
    estep = base_ap.ap[-1][0]
    return bass.AP(
        tensor=base_ap.tensor,
        offset=base_ap.offset + col0 * estep,
        ap=[pstep, [64 * estep, nseg], [estep, 64]],
    )


def build_program():
    nc = bacc.Bacc(None, target_bir_lowering=False, debug=False)

    with tile.TileContext(nc) as tc:
        ctx = ExitStack()
        with ctx:
            dram = ctx.enter_context(tc.tile_pool(name="dram", bufs=1, space="DRAM"))
            xT_d = dram.tile([ND, 128, T], BF16, kind="ExternalInput", name="xT", uniquify=False)
            wqkv_d = dram.tile([ND, 128, 772], BF16, kind="ExternalInput", name="wqkv", uniquify=False)
            wo_d = dram.tile([2, 128, D], BF16, kind="ExternalInput", name="wo", uniquify=False)
            ctab_d = dram.tile([NT, 128, 128], BF16, kind="ExternalInput", name="ctab", uniquify=False)
            stab_d = dram.tile([NT, 128, 128], BF16, kind="ExternalInput", name="stab", uniquify=False)
            mtri_d = dram.tile([128, 128], F32, kind="ExternalInput", name="mtri", uniquify=False)
            vones_d = dram.tile([128, NT, G], BF16, kind="ExternalInput", name="vones", uniquify=False)
            outp_d = dram.tile([NT, 128, D], F32, kind="ExternalOutput", name="outp", uniquify=False)

            # ---- persistent SBUF (whole kernel) ----
            persist = ctx.enter_context(tc.tile_pool(name="persist", bufs=1))
            v_all = persist.tile([128, NT, 260], BF16)     # V' natural, 4x(64+ones)
            qt01 = persist.tile([128, T], BF16)            # Q^T heads 0,1 stacked
            qt23 = persist.tile([128, T], BF16)
            kt01 = persist.tile([128, T], BF16)
            kt23 = persist.tile([128, T], BF16)
            mtri_s = persist.tile([128, 128], F32)
            stats_all = persist.tile([128, NT, 8], F32)
            rv_all = persist.tile([128, NT, 8], F32)
            eps_s = persist.tile([128, 1], F32)
            nc.vector.memset(eps_s, EPS)

            nc.sync.dma_start(out=mtri_s, in_=mtri_d)

            # ================= Phase 1: projections + rmsnorm + rope + transpose
            p1 = ExitStack()
            with p1:
                xpool = p1.enter_context(tc.tile_pool(name="xpool", bufs=1))
                wpool = p1.enter_context(tc.tile_pool(name="wpool", bufs=1))
                tabpool = p1.enter_context(tc.tile_pool(name="tabpool", bufs=1))
                work1 = p1.enter_context(tc.tile_pool(name="work1", bufs=2))
                ps_qk = p1.enter_context(tc.tile_pool(name="ps_qk", bufs=2, space="PSUM"))
                ps_v = p1.enter_context(tc.tile_pool(name="ps_v", bufs=2, space="PSUM"))

                w_s = wpool.tile([128, ND, 772], BF16)
                nc.sync.dma_start(out=w_s, in_=wqkv_d.rearrange("c p n -> p c n"))
                xt_s = []
                for c in range(ND):
                    xc = xpool.tile([128, T], BF16, name=f"xt{c}")
                    nc.sync.dma_start(out=xc, in_=xT_d[c])
                    xt_s.append(xc)
                ctab_s = tabpool.tile([128, NT, 128], BF16)
                stab_s = tabpool.tile([128, NT, 128], BF16)
                nc.sync.dma_start(out=ctab_s, in_=ctab_d.rearrange("t p n -> p t n"))
                nc.sync.dma_start(out=stab_s, in_=stab_d.rearrange("t p n -> p t n"))

                vones_cols = bass.AP(
                    tensor=v_all.tensor,
                    offset=v_all.offset + 64,
                    ap=[v_all.ap[0], [260, NT], [65, G]])
                nc.sync.dma_start(out=vones_cols, in_=vones_d)

                dsts = (qt01, qt23, kt01, kt23)
                for it in range(NT):
                    qkp = ps_qk.tile([128, 512], F32, tag="qk")
                    vp = ps_v.tile([128, 260], F32, tag="v")
                    for c in range(ND):
                        lhs = xt_s[c][:, it * 128:(it + 1) * 128]
                        nc.tensor.matmul(qkp, lhs, w_s[:, c, 0:512],
                                         start=(c == 0), stop=(c == ND - 1))
                        nc.tensor.matmul(vp, lhs, w_s[:, c, 512:772],
                                         start=(c == 0), stop=(c == ND - 1))
                    # rms stats: squares on ACT (PSUM read 1), seg-sums on DVE
                    scr = work1.tile([128, 512], BF16, tag="scr")
                    nc.scalar.activation(out=scr, in_=qkp, func=ACTF.Square)
                    nc.vector.tensor_reduce(
                        out=stats_all[:, it, :],
                        in_=scr.rearrange("p (s e) -> p s e", e=64),
                        axis=AX.X, op=ALU.add)
                    # rv = 1/sqrt(sumsq/64 + eps): ACT sqrt + fast DVE recip
                    nc.scalar.activation(out=stats_all[:, it, :], in_=stats_all[:, it, :],
                                         func=ACTF.Sqrt,
                                         scale=1.0 / HD, bias=eps_s)
                    with nc.allow_low_precision(reason="rms scale recip"):
                        nc.vector.reciprocal_approx_fast(
                            out=rv_all[:, it, :], in_=stats_all[:, it, :])
                    # fused rms apply: qhat = qkp * rv (per 64-seg), PSUM read 2
                    qhat = work1.tile([128, 512], BF16, tag="qhat")
                    rvb = bass.AP(
                        tensor=rv_all.tensor,
                        offset=rv_all[:, it, :].offset,
                        ap=[rv_all.ap[0], [1, 8], [0, 64]])
                    nc.vector.tensor_tensor(
                        out=qhat.rearrange("p (s e) -> p s e", e=64),
                        in0=qkp.rearrange("p (s e) -> p s e", e=64),
                        in1=rvb, op=ALU.mult)
                    # V drain (only the 4x64 value cols; ones cols DMA'd once)
                    vdst = bass.AP(
                        tensor=v_all.tensor,
                        offset=v_all[:, it, :].offset,
                        ap=[v_all.ap[0], [65, 4], [1, 64]])
                    vsrc = bass.AP(
                        tensor=vp.tensor,
                        offset=vp.offset,
                        ap=[vp.ap[0], [65, 4], [1, 64]])
                    nc.vector.tensor_copy(vdst, vsrc)
                    # rope: t1 = qhat*c + rot(qhat)*s  (tables have q|k halves)
                    t1 = work1.tile([128, 512], BF16, tag="t1")
                    rot = work1.tile([128, 512], BF16, tag="rot")
                    cb_view = bass.AP(
                        tensor=ctab_s.tensor,
                        offset=ctab_s[:, it, :].offset,
                        ap=[ctab_s.ap[0], [64, 2], [0, 4], [1, 64]])
                    nc.vector.tensor_tensor(
                        out=t1.rearrange("p (h r e) -> p h r e", h=2, r=4),
                        in0=qhat.rearrange("p (h r e) -> p h r e", h=2, r=4),
                        in1=cb_view, op=ALU.mult)
                    for half, c0 in ((0, 0), (1, 256)):
                        sb_view = bass.AP(
                            tensor=stab_s.tensor,
                            offset=stab_s[:, it, :].offset + 64 * half,
                            ap=[stab_s.ap[0], [0, 4], [32, 2], [1, 32]])
                        rot_out = bass.AP(
                            tensor=rot.tensor,
                            offset=rot.offset + c0,
                            ap=[rot.ap[0], [64, 4], [32, 2], [1, 32]])
                        nc.vector.tensor_tensor(
                            out=rot_out, in0=_rot_view(qhat, c0, 4),
                            in1=sb_view, op=ALU.mult)
                    nc.vector.tensor_add(out=t1, in0=t1, in1=rot)
                    # transpose 4 col-blocks -> head-major (128, t) via DMA XBAR
                    for cb in range(4):
                        nc.sync.dma_start_transpose(
                            out=dsts[cb][:, it * 128:(it + 1) * 128],
                            in_=t1[:, cb * 128:(cb + 1) * 128])

            # ====== Phases 2+3 fused: pair-packed attention + oproj bursts
            p23 = ExitStack()
            with p23:
                otpool = p23.enter_context(tc.tile_pool(name="otpool", bufs=1))
                # heads stacked per pair: rows 0-63 = even head, 64-127 = odd
                ot01 = otpool.tile([128, T], BF16)
                ot23 = otpool.tile([128, T], BF16)
                wo_s = otpool.tile([128, 2, D], BF16)  # [hv-pair rows, pair, D]
                nc.sync.dma_start(out=wo_s, in_=wo_d.rearrange("a p n -> p a n"))

                ptpool = p23.enter_context(tc.tile_pool(name="ptpool", bufs=3))
                npool = p23.enter_context(tc.tile_pool(name="npool", bufs=2))
                outpool = p23.enter_context(tc.tile_pool(name="outpool", bufs=3))
                ps_sg = p23.enter_context(tc.tile_pool(name="ps_sg", bufs=2, space="PSUM"))
                ps_o = p23.enter_context(tc.tile_pool(name="ps_o", bufs=2, space="PSUM"))

                def oproj_block(jb):
                    for it in range(4 * jb, 4 * jb + 4):
                        pso = ps_sg.tile([128, 1024], F32, tag="sg")
                        for n in range(2):
                            psl = slice(n * 512, (n + 1) * 512)
                            nc.tensor.matmul(
                                pso[:, psl],
                                ot01[:, it * 128:(it + 1) * 128],
                                wo_s[:, 0, n * 512:(n + 1) * 512],
                                start=True, stop=False)
                            nc.tensor.matmul(
                                pso[:, psl],
                                ot23[:, it * 128:(it + 1) * 128],
                                wo_s[:, 1, n * 512:(n + 1) * 512],
                                start=False, stop=True)
                        osb = outpool.tile([128, D], F32, tag="osb")
                        nc.scalar.copy(osb, pso)
                        nc.sync.dma_start(out=outp_d[it], in_=osb)

                for j in range(4):
                    kmax = 4 * (j + 1)
                    for pair in range(2):
                        qt = (qt01, qt23)[pair]
                        kt = (kt01, kt23)[pair]
                        ott = (ot01, ot23)[pair]
                        opA = ps_o.tile([65, 512], F32, tag="opA")
                        opB = ps_o.tile([65, 512], F32, tag="opB")
                        for i in range(kmax):
                            sg = ps_sg.tile([128, 1024], F32, tag="sg")
                            pt = ptpool.tile([128, 1024], BF16, tag="pt")
                            nc.tensor.matmul(
                                sg[:, 0:512],
                                kt[0:64, i * 128:(i + 1) * 128],
                                qt[0:64, j * 512:(j + 1) * 512],
                                start=True, stop=True)
                            nc.tensor.matmul(
                                sg[:, 512:1024],
                                kt[64:128, i * 128:(i + 1) * 128],
                                qt[64:128, j * 512:(j + 1) * 512],
                                start=True, stop=True)
                            r = i - 4 * j
                            c0 = max(0, 128 * r) if r >= 0 else 0
                            if r >= 0:  # diagonal block mask, both heads
                                for half in range(2):
                                    dsl = slice(half * 512 + 128 * r,
                                                half * 512 + 128 * (r + 1))
                                    nc.vector.tensor_add(
                                        out=sg[:, dsl], in0=sg[:, dsl], in1=mtri_s)
                            nc.scalar.activation(out=pt, in_=sg, func=ACTF.Exp)
                            nc.tensor.matmul(
                                opA[:, c0:512],
                                v_all[:, i, (2 * pair) * 65:(2 * pair + 1) * 65],
                                pt[:, c0:512],
                                start=(i == 0), stop=(i == kmax - 1))
                            nc.tensor.matmul(
                                opB[:, c0:512],
                                v_all[:, i, (2 * pair + 1) * 65:(2 * pair + 2) * 65],
                                pt[:, 512 + c0:1024],
                                start=(i == 0), stop=(i == kmax - 1))
                        # denominators -> recip -> DMA broadcast -> normalize
                        for half, op in ((0, opA), (1, opB)):
                            den_sb = npool.tile([1, 512], F32, tag=f"den{half}")
                            nc.vector.tensor_copy(den_sb, op[64:65, 0:512])
                            rec_sb = npool.tile([1, 512], F32, tag=f"rec{half}")
                            with nc.allow_low_precision(reason="softmax den recip"):
                                nc.vector.reciprocal_approx_fast(
                                    out=rec_sb, in_=den_sb)
                            rec64 = npool.tile([64, 512], F32, tag=f"rec64{half}")
                            rec_b = bass.AP(
                                tensor=rec_sb.tensor,
                                offset=rec_sb.offset,
                                ap=[[1, 1], [0, 64], rec_sb.ap[-1]])
                            nc.sync.dma_start(out=rec64, in_=rec_b)
                            nc.vector.tensor_mul(
                                out=ott[64 * half:64 * (half + 1),
                                        j * 512:(j + 1) * 512],
                                in0=op[0:64, 0:512], in1=rec64)
                    # oproj deferred one j-phase so drains are long done
                    if j >= 1:
                        oproj_block(j - 1)
                oproj_block(3)

    nc.compile()
    return nc


_PROGRAM = None


def _get_program():
    global _PROGRAM
    if _PROGRAM is None:
        _PROGRAM = build_program()
    return _PROGRAM


def make_inputs_for_core(core, x, Wq, Wk, Wv, Wo, q_norm_w, k_norm_w):
    bf16 = ml_dtypes.bfloat16
    b, g = core // 4, core % 4
    xT = np.ascontiguousarray(np.asarray(x[b]).T).reshape(ND, 128, T)
    wq = np.asarray(Wq)[:, 256 * g:256 * (g + 1)]
    wk = np.asarray(Wk)[:, 256 * g:256 * (g + 1)]
    wv = np.asarray(Wv)[:, 256 * g:256 * (g + 1)]
    wvp = np.zeros((D, 260), np.float32)
    for h in range(G):
        wvp[:, h * 65:h * 65 + 64] = wv[:, h * 64:(h + 1) * 64]
    wqkv = np.concatenate([wq, wk, wvp], axis=1).reshape(ND, 128, 772)
    wo = np.ascontiguousarray(
        np.asarray(Wo)[256 * g:256 * (g + 1), :].reshape(2, 128, D))

    inv_freq = 1.0 / (ROPE_BASE ** (np.arange(0, HD, 2, dtype=np.float64) / HD))
    tarr = np.arange(T, dtype=np.float64)
    fr = np.outer(tarr, inv_freq)
    cos, sin = np.cos(fr), np.sin(fr)

    def tables(w, scale):
        c = np.empty((T, HD), np.float64)
        s = np.empty((T, HD), np.float64)
        c[:, :32] = cos * w[:32] * scale
        c[:, 32:] = cos * w[32:] * scale
        s[:, :32] = -sin * w[32:] * scale
        s[:, 32:] = sin * w[:32] * scale
        return c, s

    qw = np.asarray(q_norm_w, np.float64)
    kw = np.asarray(k_norm_w, np.float64)
    cq, sq = tables(qw, 0.125)
    ck, sk = tables(kw, 1.0)
    ctab = np.concatenate([cq, ck], axis=1).reshape(NT, 128, 128)
    stab = np.concatenate([sq, sk], axis=1).reshape(NT, 128, 128)

    kp = np.arange(128)[:, None]
    qf = np.arange(128)[None, :]
    mtri = np.where(qf >= kp, 0.0, MASK_NEG).astype(np.float32)
    vones = np.ones((128, NT, G), bf16)
    return {
        "xT": xT.astype(bf16), "wqkv": wqkv.astype(bf16), "wo": wo.astype(bf16),
        "ctab": ctab.astype(bf16), "stab": stab.astype(bf16),
        "mtri": mtri, "vones": vones,
    }


def run_on_hw(inputs, trace=False):
    from concourse.bass_utils import run_bass_kernel_spmd
    nc = _get_program()
    in_maps = [make_inputs_for_core(c, **inputs) for c in range(NCORES)]
    res = run_bass_kernel_spmd(nc, in_maps, list(range(NCORES)), trace=trace)
    parts = [res.results[c]["outp"].reshape(T, D) for c in range(NCORES)]
    out = np.stack([sum(parts[0:4]), sum(parts[4:8])]).astype(np.float32)
    return out, res


def kernel(**inputs):
    out, _ = run_on_hw(inputs, trace=False)
    return out


# revision 13
# speedup vs baseline: 1.4336x; 1.0621x over previous
"""MultiHeadAttention (QK-RMSNorm + RoPE + causal) Trainium2 Bass kernel.

Sharding: 8 cores = 2 batches x 4 head-groups (4 heads each).
Each core computes a partial (2048, 1024) output (its heads' contribution
through the output projection); host sums the 4 group-partials per batch.

v2: bf16 matmul inputs (fp32 PSUM accumulation), DMA-XBAR transposes,
single-table activations, fused rms apply, on-chip softmax denominators.
"""

import math
import sys
from contextlib import ExitStack

import numpy as np
import ml_dtypes

sys.path.insert(0, "/opt/trn_rl_repo")

import concourse.bass as bass  # noqa: E402
import concourse.bacc as bacc  # noqa: E402
import concourse.tile as tile  # noqa: E402
from concourse import mybir  # noqa: E402

B = 2
T = 2048
D = 1024
H = 16
HD = 64
G = 4  # heads per core
NCORES = 8
NT = T // 128  # 16 t-tiles
ND = D // 128  # 8 d-chunks
EPS = 1e-6
ROPE_BASE = 10000.0
MASK_NEG = -30000.0

F32 = mybir.dt.float32
BF16 = mybir.dt.bfloat16
AX = mybir.AxisListType
ALU = mybir.AluOpType
ACTF = mybir.ActivationFunctionType


def _rot_view(base_ap, col0, nseg):
    """View of base_ap cols [col0, col0+64*nseg) with halves swapped per
    64-wide segment: (P, nseg, 2, 32) reading [32:64] then [0:32]."""
    pstep = base_ap.ap[0]
    estep = base_ap.ap[-1][0]
    return bass.AP(
        tensor=base_ap.tensor,
        offset=base_ap.offset + (col0 + 32) * estep,
        ap=[pstep, [64 * estep, nseg], [-32 * estep, 2], [estep, 32]],
    )


def _seg_view(base_ap, col0, nseg):
    """(P, nseg, 64) view of base_ap cols [col0, col0+64*nseg)."""
    pstep = base_ap.ap[0]
    estep = base_ap.ap[-1][0]
    return bass.AP(
        tensor=base_ap.tensor,
        offset=base_ap.offset + col0 * estep,
        ap=[pstep, [64 * estep, nseg], [estep, 64]],
    )


def build_program():
    nc = bacc.Bacc(None, target_bir_lowering=False, debug=False)

    with tile.TileContext(nc) as tc:
        ctx = ExitStack()
        with ctx:
            dram = ctx.enter_context(tc.tile_pool(name="dram", bufs=1, space="DRAM"))
            xT_d = dram.tile([ND, 128, T], BF16, kind="ExternalInput", name="xT", uniquify=False)
            wqkv_d = dram.tile([ND, 128, 772], BF16, kind="ExternalInput", name="wqkv", uniquify=False)
            wo_d = dram.tile([2, 128, D], BF16, kind="ExternalInput", name="wo", uniquify=False)
            ctab_d = dram.tile([NT, 128, 128], BF16, kind="ExternalInput", name="ctab", uniquify=False)
            stab_d = dram.tile([NT, 128, 128], BF16, kind="ExternalInput", name="stab", uniquify=False)
            mtri_d = dram.tile([128, 128], F32, kind="ExternalInput", name="mtri", uniquify=False)
            vones_d = dram.tile([128, NT, G], BF16, kind="ExternalInput", name="vones", uniquify=False)
            outp_d = dram.tile([NT, 128, D], F32, kind="ExternalOutput", name="outp", uniquify=False)

            # ---- persistent SBUF (whole kernel) ----
            persist = ctx.enter_context(tc.tile_pool(name="persist", bufs=1))
            v_all = persist.tile([128, NT, 260], BF16)     # V' natural, 4x(64+ones)
            qt01 = persist.tile([128, T], BF16)            # Q^T heads 0,1 stacked
            qt23 = persist.tile([128, T], BF16)
            kt01 = persist.tile([128, T], BF16)
            kt23 = persist.tile([128, T], BF16)
            mtri_s = persist.tile([128, 128], F32)
            stats_all = persist.tile([128, NT, 8], F32)
            rv_all = persist.tile([128, NT, 8], F32)
            eps_s = persist.tile([128, 1], F32)
            nc.vector.memset(eps_s, EPS)

            nc.sync.dma_start(out=mtri_s, in_=mtri_d)

            # ================= Phase 1: projections + rmsnorm + rope + transpose
            p1 = ExitStack()
            with p1:
                xpool = p1.enter_context(tc.tile_pool(name="xpool", bufs=1))
                wpool = p1.enter_context(tc.tile_pool(name="wpool", bufs=1))
                tabpool = p1.enter_context(tc.tile_pool(name="tabpool", bufs=1))
                work1 = p1.enter_context(tc.tile_pool(name="work1", bufs=2))
                ps_qk = p1.enter_context(tc.tile_pool(name="ps_qk", bufs=2, space="PSUM"))
                ps_v = p1.enter_context(tc.tile_pool(name="ps_v", bufs=2, space="PSUM"))

                w_s = wpool.tile([128, ND, 772], BF16)
                nc.sync.dma_start(out=w_s, in_=wqkv_d.rearrange("c p n -> p c n"))
                xt_s = []
                for c in range(ND):
                    xc = xpool.tile([128, T], BF16, name=f"xt{c}")
                    nc.sync.dma_start(out=xc, in_=xT_d[c])
                    xt_s.append(xc)
                ctab_s = tabpool.tile([128, NT, 128], BF16)
                stab_s = tabpool.tile([128, NT, 128], BF16)
                nc.sync.dma_start(out=ctab_s, in_=ctab_d.rearrange("t p n -> p t n"))
                nc.sync.dma_start(out=stab_s, in_=stab_d.rearrange("t p n -> p t n"))

                vones_cols = bass.AP(
                    tensor=v_all.tensor,
                    offset=v_all.offset + 64,
                    ap=[v_all.ap[0], [260, NT], [65, G]])
                nc.sync.dma_start(out=vones_cols, in_=vones_d)

                dsts = (qt01, qt23, kt01, kt23)
                for it in range(NT):
                    qkp = ps_qk.tile([128, 512], F32, tag="qk")
                    vp = ps_v.tile([128, 260], F32, tag="v")
                    for c in range(ND):
                        lhs = xt_s[c][:, it * 128:(it + 1) * 128]
                        nc.tensor.matmul(qkp, lhs, w_s[:, c, 0:512],
                                         start=(c == 0), stop=(c == ND - 1))
                        nc.tensor.matmul(vp, lhs, w_s[:, c, 512:772],
                                         start=(c == 0), stop=(c == ND - 1))
                    # rms stats: squares on ACT (PSUM read 1), seg-sums on DVE
                    scr = work1.tile([128, 512], BF16, tag="scr")
                    nc.scalar.activation(out=scr, in_=qkp, func=ACTF.Square)
                    nc.vector.tensor_reduce(
                        out=stats_all[:, it, :],
                        in_=scr.rearrange("p (s e) -> p s e", e=64),
                        axis=AX.X, op=ALU.add)
                    # rv = 1/sqrt(sumsq/64 + eps): ACT sqrt + fast DVE recip
                    nc.scalar.activation(out=stats_all[:, it, :], in_=stats_all[:, it, :],
                                         func=ACTF.Sqrt,
                                         scale=1.0 / HD, bias=eps_s)
                    with nc.allow_low_precision(reason="rms scale recip"):
                        nc.vector.reciprocal_approx_fast(
                            out=rv_all[:, it, :], in_=stats_all[:, it, :])
                    # fused rms apply: qhat = qkp * rv (per 64-seg), PSUM read 2
                    qhat = work1.tile([128, 512], BF16, tag="qhat")
                    rvb = bass.AP(
                        tensor=rv_all.tensor,
                        offset=rv_all[:, it, :].offset,
                        ap=[rv_all.ap[0], [1, 8], [0, 64]])
                    nc.vector.tensor_tensor(
                        out=qhat.rearrange("p (s e) -> p s e", e=64),
                        in0=qkp.rearrange("p (s e) -> p s e", e=64),
                        in1=rvb, op=ALU.mult)
                    # V drain (only the 4x64 value cols; ones cols DMA'd once)
                    vdst = bass.AP(
                        tensor=v_all.tensor,
                        offset=v_all[:, it, :].offset,
                        ap=[v_all.ap[0], [65, 4], [1, 64]])
                    vsrc = bass.AP(
                        tensor=vp.tensor,
                        offset=vp.offset,
                        ap=[vp.ap[0], [65, 4], [1, 64]])
                    nc.vector.tensor_copy(vdst, vsrc)
                    # rope: t1 = qhat*c + rot(qhat)*s  (tables have q|k halves)
                    t1 = work1.tile([128, 512], BF16, tag="t1")
                    rot = work1.tile([128, 512], BF16, tag="rot")
                    cb_view = bass.AP(
                        tensor=ctab_s.tensor,
                        offset=ctab_s[:, it, :].offset,
                        ap=[ctab_s.ap[0], [64, 2], [0, 4], [1, 64]])
                    nc.vector.tensor_tensor(
                        out=t1.rearrange("p (h r e) -> p h r e", h=2, r=4),
                        in0=qhat.rearrange("p (h r e) -> p h r e", h=2, r=4),
                        in1=cb_view, op=ALU.mult)
                    for half, c0 in ((0, 0), (1, 256)):
                        sb_view = bass.AP(
                            tensor=stab_s.tensor,
                            offset=stab_s[:, it, :].offset + 64 * half,
                            ap=[stab_s.ap[0], [0, 4], [32, 2], [1, 32]])
                        rot_out = bass.AP(
                            tensor=rot.tensor,
                            offset=rot.offset + c0,
                            ap=[rot.ap[0], [64, 4], [32, 2], [1, 32]])
                        nc.vector.tensor_tensor(
                            out=rot_out, in0=_rot_view(qhat, c0, 4),
                            in1=sb_view, op=ALU.mult)
                    nc.vector.tensor_add(out=t1, in0=t1, in1=rot)
                    # transpose 4 col-blocks -> head-major (128, t) via DMA XBAR
                    for cb in range(4):
                        nc.sync.dma_start_transpose(
                            out=dsts[cb][:, it * 128:(it + 1) * 128],
                            in_=t1[:, cb * 128:(cb + 1) * 128])

            # ====== Phases 2+3 fused: pair-packed attention + oproj bursts
            p23 = ExitStack()
            with p23:
                otpool = p23.enter_context(tc.tile_pool(name="otpool", bufs=1))
                # heads stacked per pair: rows 0-63 = even head, 64-127 = odd
                ot01 = otpool.tile([128, T], BF16)
                ot23 = otpool.tile([128, T], BF16)
                wo_s = otpool.tile([128, 2, D], BF16)  # [hv-pair rows, pair, D]
                nc.sync.dma_start(out=wo_s, in_=wo_d.rearrange("a p n -> p a n"))

                ptpool = p23.enter_context(tc.tile_pool(name="ptpool", bufs=3))
                npool = p23.enter_context(tc.tile_pool(name="npool", bufs=2))
                outpool = p23.enter_context(tc.tile_pool(name="outpool", bufs=3))
                ps_sg = p23.enter_context(tc.tile_pool(name="ps_sg", bufs=2, space="PSUM"))
                ps_o = p23.enter_context(tc.tile_pool(name="ps_o", bufs=2, space="PSUM"))

                def oproj_block(jb):
                    for it in range(4 * jb, 4 * jb + 4):
                        pso = ps_sg.tile([128, 1024], F32, tag="sg")
                        for n in range(2):
                            psl = slice(n * 512, (n + 1) * 512)
                            nc.tensor.matmul(
                                pso[:, psl],
                                ot01[:, it * 128:(it + 1) * 128],
                                wo_s[:, 0, n * 512:(n + 1) * 512],
                                start=True, stop=False)
                            nc.tensor.matmul(
                                pso[:, psl],
                                ot23[:, it * 128:(it + 1) * 128],
                                wo_s[:, 1, n * 512:(n + 1) * 512],
                                start=False, stop=True)
                        osb = outpool.tile([128, D], F32, tag="osb")
                        nc.scalar.copy(osb, pso)
                        nc.sync.dma_start(out=outp_d[it], in_=osb)

                pending = []  # deferred normalize muls: (op, rec64, ott, half, j)

                def flush_pending():
                    for op_, rec64_, ott_, half_, j_ in pending:
                        nc.vector.tensor_mul(
                            out=ott_[64 * half_:64 * (half_ + 1),
                                     j_ * 512:(j_ + 1) * 512],
                            in0=op_[0:64, 0:512], in1=rec64_)
                    pending.clear()

                for j in range(4):
                    kmax = 4 * (j + 1)
                    for pair in range(2):
                        qt = (qt01, qt23)[pair]
                        kt = (kt01, kt23)[pair]
                        ott = (ot01, ot23)[pair]
                        opA = ps_o.tile([65, 512], F32, tag="opA")
                        opB = ps_o.tile([65, 512], F32, tag="opB")
                        for i in range(kmax):
                            if i == 1:
                                flush_pending()
                            sg = ps_sg.tile([128, 1024], F32, tag="sg")
                            pt = ptpool.tile([128, 1024], BF16, tag="pt")
                            nc.tensor.matmul(
                                sg[:, 0:512],
                                kt[0:64, i * 128:(i + 1) * 128],
                                qt[0:64, j * 512:(j + 1) * 512],
                                start=True, stop=True)
                            nc.tensor.matmul(
                                sg[:, 512:1024],
                                kt[64:128, i * 128:(i + 1) * 128],
                                qt[64:128, j * 512:(j + 1) * 512],
                                start=True, stop=True)
                            r = i - 4 * j
                            c0 = max(0, 128 * r) if r >= 0 else 0
                            if r >= 0:  # diagonal block mask, both heads
                                for half in range(2):
                                    dsl = slice(half * 512 + 128 * r,
                                                half * 512 + 128 * (r + 1))
                                    nc.vector.tensor_add(
                                        out=sg[:, dsl], in0=sg[:, dsl], in1=mtri_s)
                            nc.scalar.activation(out=pt, in_=sg, func=ACTF.Exp)
                            nc.tensor.matmul(
                                opA[:, c0:512],
                                v_all[:, i, (2 * pair) * 65:(2 * pair + 1) * 65],
                                pt[:, c0:512],
                                start=(i == 0), stop=(i == kmax - 1))
                            nc.tensor.matmul(
                                opB[:, c0:512],
                                v_all[:, i, (2 * pair + 1) * 65:(2 * pair + 2) * 65],
                                pt[:, 512 + c0:1024],
                                start=(i == 0), stop=(i == kmax - 1))
                        # denominators -> recip -> DMA broadcast; mul deferred
                        for half, op in ((0, opA), (1, opB)):
                            den_sb = npool.tile([1, 512], F32, tag=f"den{half}")
                            nc.vector.tensor_copy(den_sb, op[64:65, 0:512])
                            rec_sb = npool.tile([1, 512], F32, tag=f"rec{half}")
                            with nc.allow_low_precision(reason="softmax den recip"):
                                nc.vector.reciprocal_approx_fast(
                                    out=rec_sb, in_=den_sb)
                            rec64 = npool.tile([64, 512], F32, tag=f"rec64{half}")
                            rec_b = bass.AP(
                                tensor=rec_sb.tensor,
                                offset=rec_sb.offset,
                                ap=[[1, 1], [0, 64], rec_sb.ap[-1]])
                            nc.sync.dma_start(out=rec64, in_=rec_b)
                            pending.append((op, rec64, ott, half, j))
                    # oproj deferred one j-phase so drains are long done
                    if j >= 1:
                        oproj_block(j - 1)
                flush_pending()
                oproj_block(3)

    nc.compile()
    return nc


_PROGRAM = None


def _get_program():
    global _PROGRAM
    if _PROGRAM is None:
        _PROGRAM = build_program()
    return _PROGRAM


def make_inputs_for_core(core, x, Wq, Wk, Wv, Wo, q_norm_w, k_norm_w):
    bf16 = ml_dtypes.bfloat16
    b, g = core // 4, core % 4
    xT = np.ascontiguousarray(np.asarray(x[b]).T).reshape(ND, 128, T)
    wq = np.asarray(Wq)[:, 256 * g:256 * (g + 1)]
    wk = np.asarray(Wk)[:, 256 * g:256 * (g + 1)]
    wv = np.asarray(Wv)[:, 256 * g:256 * (g + 1)]
    wvp = np.zeros((D, 260), np.float32)
    for h in range(G):
        wvp[:, h * 65:h * 65 + 64] = wv[:, h * 64:(h + 1) * 64]
    wqkv = np.concatenate([wq, wk, wvp], axis=1).reshape(ND, 128, 772)
    wo = np.ascontiguousarray(
        np.asarray(Wo)[256 * g:256 * (g + 1), :].reshape(2, 128, D))

    inv_freq = 1.0 / (ROPE_BASE ** (np.arange(0, HD, 2, dtype=np.float64) / HD))
    tarr = np.arange(T, dtype=np.float64)
    fr = np.outer(tarr, inv_freq)
    cos, sin = np.cos(fr), np.sin(fr)

    def tables(w, scale):
        c = np.empty((T, HD), np.float64)
        s = np.empty((T, HD), np.float64)
        c[:, :32] = cos * w[:32] * scale
        c[:, 32:] = cos * w[32:] * scale
        s[:, :32] = -sin * w[32:] * scale
        s[:, 32:] = sin * w[:32] * scale
        return c, s

    qw = np.asarray(q_norm_w, np.float64)
    kw = np.asarray(k_norm_w, np.float64)
    cq, sq = tables(qw, 0.125)
    ck, sk = tables(kw, 1.0)
    ctab = np.concatenate([cq, ck], axis=1).reshape(NT, 128, 128)
    stab = np.concatenate([sq, sk], axis=1).reshape(NT, 128, 128)

    kp = np.arange(128)[:, None]
    qf = np.arange(128)[None, :]
    mtri = np.where(qf >= kp, 0.0, MASK_NEG).astype(np.float32)
    vones = np.ones((128, NT, G), bf16)
    return {
        "xT": xT.astype(bf16), "wqkv": wqkv.astype(bf16), "wo": wo.astype(bf16),
        "ctab": ctab.astype(bf16), "stab": stab.astype(bf16),
        "mtri": mtri, "vones": vones,
    }


def run_on_hw(inputs, trace=False):
    from concourse.bass_utils import run_bass_kernel_spmd
    nc = _get_program()
    in_maps = [make_inputs_for_core(c, **inputs) for c in range(NCORES)]
    res = run_bass_kernel_spmd(nc, in_maps, list(range(NCORES)), trace=trace)
    parts = [res.results[c]["outp"].reshape(T, D) for c in range(NCORES)]
    out = np.stack([sum(parts[0:4]), sum(parts[4:8])]).astype(np.float32)
    return out, res


def kernel(**inputs):
    out, _ = run_on_hw(inputs, trace=False)
    return out


# revision 15
# speedup vs baseline: 1.4491x; 1.0108x over previous
"""MultiHeadAttention (QK-RMSNorm + RoPE + causal) Trainium2 Bass kernel.

Sharding: 8 cores = 2 batches x 4 head-groups (4 heads each).
Each core computes a partial (2048, 1024) output (its heads' contribution
through the output projection); host sums the 4 group-partials per batch.

v2: bf16 matmul inputs (fp32 PSUM accumulation), DMA-XBAR transposes,
single-table activations, fused rms apply, on-chip softmax denominators.
"""

import math
import sys
from contextlib import ExitStack

import numpy as np
import ml_dtypes

sys.path.insert(0, "/opt/trn_rl_repo")

import concourse.bass as bass  # noqa: E402
import concourse.bacc as bacc  # noqa: E402
import concourse.tile as tile  # noqa: E402
from concourse import mybir  # noqa: E402

B = 2
T = 2048
D = 1024
H = 16
HD = 64
G = 4  # heads per core
NCORES = 8
NT = T // 128  # 16 t-tiles
ND = D // 128  # 8 d-chunks
EPS = 1e-6
ROPE_BASE = 10000.0
MASK_NEG = -30000.0

F32 = mybir.dt.float32
BF16 = mybir.dt.bfloat16
AX = mybir.AxisListType
ALU = mybir.AluOpType
ACTF = mybir.ActivationFunctionType


def _rot_view(base_ap, col0, nseg):
    """View of base_ap cols [col0, col0+64*nseg) with halves swapped per
    64-wide segment: (P, nseg, 2, 32) reading [32:64] then [0:32]."""
    pstep = base_ap.ap[0]
    estep = base_ap.ap[-1][0]
    return bass.AP(
        tensor=base_ap.tensor,
        offset=base_ap.offset + (col0 + 32) * estep,
        ap=[pstep, [64 * estep, nseg], [-32 * estep, 2], [estep, 32]],
    )


def _seg_view(base_ap, col0, nseg):
    """(P, nseg, 64) view of base_ap cols [col0, col0+64*nseg)."""
    pstep = base_ap.ap[0]
    estep = base_ap.ap[-1][0]
    return bass.AP(
        tensor=base_ap.tensor,
        offset=base_ap.offset + col0 * estep,
        ap=[pstep, [64 * estep, nseg], [estep, 64]],
    )


def build_program():
    nc = bacc.Bacc(None, target_bir_lowering=False, debug=False)

    with tile.TileContext(nc) as tc:
        ctx = ExitStack()
        with ctx:
            dram = ctx.enter_context(tc.tile_pool(name="dram", bufs=1, space="DRAM"))
            xT_d = dram.tile([ND, 128, T], BF16, kind="ExternalInput", name="xT", uniquify=False)
            wqkv_d = dram.tile([ND, 128, 772], BF16, kind="ExternalInput", name="wqkv", uniquify=False)
            wo_d = dram.tile([2, 128, D], BF16, kind="ExternalInput", name="wo", uniquify=False)
            ctab_d = dram.tile([NT, 128, 128], BF16, kind="ExternalInput", name="ctab", uniquify=False)
            stab_d = dram.tile([NT, 128, 128], BF16, kind="ExternalInput", name="stab", uniquify=False)
            mtri_d = dram.tile([128, 128], F32, kind="ExternalInput", name="mtri", uniquify=False)
            vones_d = dram.tile([128, NT, G], BF16, kind="ExternalInput", name="vones", uniquify=False)
            outp_d = dram.tile([NT, 128, D], F32, kind="ExternalOutput", name="outp", uniquify=False)

            # ---- persistent SBUF (whole kernel) ----
            persist = ctx.enter_context(tc.tile_pool(name="persist", bufs=1))
            v_all = persist.tile([128, NT, 260], BF16)     # V' natural, 4x(64+ones)
            qt01 = persist.tile([128, T], BF16)            # Q^T heads 0,1 stacked
            qt23 = persist.tile([128, T], BF16)
            kt01 = persist.tile([128, T], BF16)
            kt23 = persist.tile([128, T], BF16)
            mtri_s = persist.tile([128, 128], F32)
            stats_all = persist.tile([128, NT, 8], F32)
            rv_all = persist.tile([128, NT, 8], F32)
            eps_s = persist.tile([128, 1], F32)
            nc.vector.memset(eps_s, EPS)

            nc.sync.dma_start(out=mtri_s, in_=mtri_d)

            # ================= Phase 1: projections + rmsnorm + rope + transpose
            p1 = ExitStack()
            with p1:
                xpool = p1.enter_context(tc.tile_pool(name="xpool", bufs=1))
                wpool = p1.enter_context(tc.tile_pool(name="wpool", bufs=1))
                tabpool = p1.enter_context(tc.tile_pool(name="tabpool", bufs=1))
                work1 = p1.enter_context(tc.tile_pool(name="work1", bufs=2))
                ps_qk = p1.enter_context(tc.tile_pool(name="ps_qk", bufs=2, space="PSUM"))
                ps_v = p1.enter_context(tc.tile_pool(name="ps_v", bufs=2, space="PSUM"))

                w_s = wpool.tile([128, ND, 772], BF16)
                xt_s = []
                for c in range(ND):
                    xc = xpool.tile([128, T], BF16, name=f"xt{c}")
                    nc.sync.dma_start(out=xc, in_=xT_d[c])
                    nc.sync.dma_start(out=w_s[:, c, :], in_=wqkv_d[c])
                    xt_s.append(xc)
                ctab_s = tabpool.tile([128, NT, 128], BF16)
                stab_s = tabpool.tile([128, NT, 128], BF16)
                nc.sync.dma_start(out=ctab_s, in_=ctab_d.rearrange("t p n -> p t n"))
                nc.sync.dma_start(out=stab_s, in_=stab_d.rearrange("t p n -> p t n"))

                vones_cols = bass.AP(
                    tensor=v_all.tensor,
                    offset=v_all.offset + 64,
                    ap=[v_all.ap[0], [260, NT], [65, G]])
                nc.sync.dma_start(out=vones_cols, in_=vones_d)

                dsts = (qt01, qt23, kt01, kt23)
                for it in range(NT):
                    qkp = ps_qk.tile([128, 512], F32, tag="qk")
                    vp = ps_v.tile([128, 260], F32, tag="v")
                    for c in range(ND):
                        lhs = xt_s[c][:, it * 128:(it + 1) * 128]
                        nc.tensor.matmul(qkp, lhs, w_s[:, c, 0:512],
                                         start=(c == 0), stop=(c == ND - 1))
                        nc.tensor.matmul(vp, lhs, w_s[:, c, 512:772],
                                         start=(c == 0), stop=(c == ND - 1))
                    # rms stats: squares on ACT (PSUM read 1), seg-sums on DVE
                    scr = work1.tile([128, 512], BF16, tag="scr")
                    nc.scalar.activation(out=scr, in_=qkp, func=ACTF.Square)
                    nc.vector.tensor_reduce(
                        out=stats_all[:, it, :],
                        in_=scr.rearrange("p (s e) -> p s e", e=64),
                        axis=AX.X, op=ALU.add)
                    # rv = 1/sqrt(sumsq/64 + eps): ACT sqrt + fast DVE recip
                    nc.scalar.activation(out=stats_all[:, it, :], in_=stats_all[:, it, :],
                                         func=ACTF.Sqrt,
                                         scale=1.0 / HD, bias=eps_s)
                    with nc.allow_low_precision(reason="rms scale recip"):
                        nc.vector.reciprocal_approx_fast(
                            out=rv_all[:, it, :], in_=stats_all[:, it, :])
                    # fused rms apply: qhat = qkp * rv (per 64-seg), PSUM read 2
                    qhat = work1.tile([128, 512], BF16, tag="qhat")
                    rvb = bass.AP(
                        tensor=rv_all.tensor,
                        offset=rv_all[:, it, :].offset,
                        ap=[rv_all.ap[0], [1, 8], [0, 64]])
                    nc.vector.tensor_tensor(
                        out=qhat.rearrange("p (s e) -> p s e", e=64),
                        in0=qkp.rearrange("p (s e) -> p s e", e=64),
                        in1=rvb, op=ALU.mult)
                    # V drain (only the 4x64 value cols; ones cols DMA'd once)
                    vdst = bass.AP(
                        tensor=v_all.tensor,
                        offset=v_all[:, it, :].offset,
                        ap=[v_all.ap[0], [65, 4], [1, 64]])
                    vsrc = bass.AP(
                        tensor=vp.tensor,
                        offset=vp.offset,
                        ap=[vp.ap[0], [65, 4], [1, 64]])
                    nc.scalar.copy(vdst, vsrc)
                    # rope: t1 = qhat*c + rot(qhat)*s  (tables have q|k halves)
                    t1 = work1.tile([128, 512], BF16, tag="t1")
                    rot = work1.tile([128, 512], BF16, tag="rot")
                    cb_view = bass.AP(
                        tensor=ctab_s.tensor,
                        offset=ctab_s[:, it, :].offset,
                        ap=[ctab_s.ap[0], [64, 2], [0, 4], [1, 64]])
                    nc.vector.tensor_tensor(
                        out=t1.rearrange("p (h r e) -> p h r e", h=2, r=4),
                        in0=qhat.rearrange("p (h r e) -> p h r e", h=2, r=4),
                        in1=cb_view, op=ALU.mult)
                    for half, c0 in ((0, 0), (1, 256)):
                        sb_view = bass.AP(
                            tensor=stab_s.tensor,
                            offset=stab_s[:, it, :].offset + 64 * half,
                            ap=[stab_s.ap[0], [0, 4], [32, 2], [1, 32]])
                        rot_out = bass.AP(
                            tensor=rot.tensor,
                            offset=rot.offset + c0,
                            ap=[rot.ap[0], [64, 4], [32, 2], [1, 32]])
                        nc.vector.tensor_tensor(
                            out=rot_out, in0=_rot_view(qhat, c0, 4),
                            in1=sb_view, op=ALU.mult)
                    nc.vector.tensor_add(out=t1, in0=t1, in1=rot)
                    # transpose 4 col-blocks -> head-major (128, t) via DMA XBAR
                    for cb in range(4):
                        nc.sync.dma_start_transpose(
                            out=dsts[cb][:, it * 128:(it + 1) * 128],
                            in_=t1[:, cb * 128:(cb + 1) * 128])

            # ====== Phases 2+3 fused: pair-packed attention + oproj bursts
            p23 = ExitStack()
            with p23:
                otpool = p23.enter_context(tc.tile_pool(name="otpool", bufs=1))
                # heads stacked per pair: rows 0-63 = even head, 64-127 = odd
                ot01 = otpool.tile([128, T], BF16)
                ot23 = otpool.tile([128, T], BF16)
                wo_s = otpool.tile([128, 2, D], BF16)  # [hv-pair rows, pair, D]
                nc.sync.dma_start(out=wo_s, in_=wo_d.rearrange("a p n -> p a n"))

                ptpool = p23.enter_context(tc.tile_pool(name="ptpool", bufs=3))
                npool = p23.enter_context(tc.tile_pool(name="npool", bufs=2))
                outpool = p23.enter_context(tc.tile_pool(name="outpool", bufs=3))
                ps_sg = p23.enter_context(tc.tile_pool(name="ps_sg", bufs=2, space="PSUM"))
                ps_o = p23.enter_context(tc.tile_pool(name="ps_o", bufs=2, space="PSUM"))

                def oproj_block(jb):
                    for it in range(4 * jb, 4 * jb + 4):
                        pso = ps_sg.tile([128, 1024], F32, tag="sg")
                        for n in range(2):
                            psl = slice(n * 512, (n + 1) * 512)
                            nc.tensor.matmul(
                                pso[:, psl],
                                ot01[:, it * 128:(it + 1) * 128],
                                wo_s[:, 0, n * 512:(n + 1) * 512],
                                start=True, stop=False)
                            nc.tensor.matmul(
                                pso[:, psl],
                                ot23[:, it * 128:(it + 1) * 128],
                                wo_s[:, 1, n * 512:(n + 1) * 512],
                                start=False, stop=True)
                        osb = outpool.tile([128, D], F32, tag="osb")
                        nc.scalar.copy(osb, pso)
                        nc.sync.dma_start(out=outp_d[it], in_=osb)

                pending = []  # deferred normalize muls: (op, rec64, ott, half, j)

                def flush_pending():
                    for op_, rec64_, ott_, half_, j_ in pending:
                        nc.vector.tensor_mul(
                            out=ott_[64 * half_:64 * (half_ + 1),
                                     j_ * 512:(j_ + 1) * 512],
                            in0=op_[0:64, 0:512], in1=rec64_)
                    pending.clear()

                for j in range(4):
                    kmax = 4 * (j + 1)
                    for pair in range(2):
                        qt = (qt01, qt23)[pair]
                        kt = (kt01, kt23)[pair]
                        ott = (ot01, ot23)[pair]
                        opA = ps_o.tile([65, 512], F32, tag="opA")
                        opB = ps_o.tile([65, 512], F32, tag="opB")
                        for i in range(kmax):
                            if i == 1:
                                flush_pending()
                            sg = ps_sg.tile([128, 1024], F32, tag="sg")
                            pt = ptpool.tile([128, 1024], BF16, tag="pt")
                            nc.tensor.matmul(
                                sg[:, 0:512],
                                kt[0:64, i * 128:(i + 1) * 128],
                                qt[0:64, j * 512:(j + 1) * 512],
                                start=True, stop=True)
                            nc.tensor.matmul(
                                sg[:, 512:1024],
                                kt[64:128, i * 128:(i + 1) * 128],
                                qt[64:128, j * 512:(j + 1) * 512],
                                start=True, stop=True)
                            r = i - 4 * j
                            c0 = max(0, 128 * r) if r >= 0 else 0
                            if r >= 0:  # diagonal block mask, both heads
                                for half in range(2):
                                    dsl = slice(half * 512 + 128 * r,
                                                half * 512 + 128 * (r + 1))
                                    nc.vector.tensor_add(
                                        out=sg[:, dsl], in0=sg[:, dsl], in1=mtri_s)
                            nc.scalar.activation(out=pt, in_=sg, func=ACTF.Exp)
                            nc.tensor.matmul(
                                opA[:, c0:512],
                                v_all[:, i, (2 * pair) * 65:(2 * pair + 1) * 65],
                                pt[:, c0:512],
                                start=(i == 0), stop=(i == kmax - 1))
                            nc.tensor.matmul(
                                opB[:, c0:512],
                                v_all[:, i, (2 * pair + 1) * 65:(2 * pair + 2) * 65],
                                pt[:, 512 + c0:1024],
                                start=(i == 0), stop=(i == kmax - 1))
                        # denominators -> recip -> DMA broadcast; mul deferred
                        for half, op in ((0, opA), (1, opB)):
                            den_sb = npool.tile([1, 512], F32, tag=f"den{half}")
                            nc.vector.tensor_copy(den_sb, op[64:65, 0:512])
                            rec_sb = npool.tile([1, 512], F32, tag=f"rec{half}")
                            with nc.allow_low_precision(reason="softmax den recip"):
                                nc.vector.reciprocal_approx_fast(
                                    out=rec_sb, in_=den_sb)
                            rec64 = npool.tile([64, 512], F32, tag=f"rec64{half}")
                            rec_b = bass.AP(
                                tensor=rec_sb.tensor,
                                offset=rec_sb.offset,
                                ap=[[1, 1], [0, 64], rec_sb.ap[-1]])
                            nc.sync.dma_start(out=rec64, in_=rec_b)
                            pending.append((op, rec64, ott, half, j))
                    # oproj deferred one j-phase so drains are long done
                    if j >= 1:
                        oproj_block(j - 1)
                flush_pending()
                oproj_block(3)

    nc.compile()
    return nc


_PROGRAM = None


def _get_program():
    global _PROGRAM
    if _PROGRAM is None:
        _PROGRAM = build_program()
    return _PROGRAM


def make_inputs_for_core(core, x, Wq, Wk, Wv, Wo, q_norm_w, k_norm_w):
    bf16 = ml_dtypes.bfloat16
    b, g = core // 4, core % 4
    xT = np.ascontiguousarray(np.asarray(x[b]).T).reshape(ND, 128, T)
    wq = np.asarray(Wq)[:, 256 * g:256 * (g + 1)]
    wk = np.asarray(Wk)[:, 256 * g:256 * (g + 1)]
    wv = np.asarray(Wv)[:, 256 * g:256 * (g + 1)]
    wvp = np.zeros((D, 260), np.float32)
    for h in range(G):
        wvp[:, h * 65:h * 65 + 64] = wv[:, h * 64:(h + 1) * 64]
    wqkv = np.concatenate([wq, wk, wvp], axis=1).reshape(ND, 128, 772)
    wo = np.ascontiguousarray(
        np.asarray(Wo)[256 * g:256 * (g + 1), :].reshape(2, 128, D))

    inv_freq = 1.0 / (ROPE_BASE ** (np.arange(0, HD, 2, dtype=np.float64) / HD))
    tarr = np.arange(T, dtype=np.float64)
    fr = np.outer(tarr, inv_freq)
    cos, sin = np.cos(fr), np.sin(fr)

    def tables(w, scale):
        c = np.empty((T, HD), np.float64)
        s = np.empty((T, HD), np.float64)
        c[:, :32] = cos * w[:32] * scale
        c[:, 32:] = cos * w[32:] * scale
        s[:, :32] = -sin * w[32:] * scale
        s[:, 32:] = sin * w[:32] * scale
        return c, s

    qw = np.asarray(q_norm_w, np.float64)
    kw = np.asarray(k_norm_w, np.float64)
    cq, sq = tables(qw, 0.125)
    ck, sk = tables(kw, 1.0)
    ctab = np.concatenate([cq, ck], axis=1).reshape(NT, 128, 128)
    stab = np.concatenate([sq, sk], axis=1).reshape(NT, 128, 128)

    kp = np.arange(128)[:, None]
    qf = np.arange(128)[None, :]
    mtri = np.where(qf >= kp, 0.0, MASK_NEG).astype(np.float32)
    vones = np.ones((128, NT, G), bf16)
    return {
        "xT": xT.astype(bf16), "wqkv": wqkv.astype(bf16), "wo": wo.astype(bf16),
        "ctab": ctab.astype(bf16), "stab": stab.astype(bf16),
        "mtri": mtri, "vones": vones,
    }


def run_on_hw(inputs, trace=False):
    from concourse.bass_utils import run_bass_kernel_spmd
    nc = _get_program()
    in_maps = [make_inputs_for_core(c, **inputs) for c in range(NCORES)]
    res = run_bass_kernel_spmd(nc, in_maps, list(range(NCORES)), trace=trace)
    parts = [res.results[c]["outp"].reshape(T, D) for c in range(NCORES)]
    out = np.stack([sum(parts[0:4]), sum(parts[4:8])]).astype(np.float32)
    return out, res


def kernel(**inputs):
    out, _ = run_on_hw(inputs, trace=False)
    return out


# revision 21
# speedup vs baseline: 1.5675x; 1.0817x over previous
"""MultiHeadAttention (QK-RMSNorm + RoPE + causal) Trainium2 Bass kernel.

Sharding: 8 cores = 2 batches x 4 head-groups (4 heads each).
Each core computes a partial (2048, 1024) output (its heads' contribution
through the output projection); host sums the 4 group-partials per batch.

v2: bf16 matmul inputs (fp32 PSUM accumulation), DMA-XBAR transposes,
single-table activations, fused rms apply, on-chip softmax denominators.
"""

import math
import sys
from contextlib import ExitStack

import numpy as np
import ml_dtypes

sys.path.insert(0, "/opt/trn_rl_repo")

import concourse.bass as bass  # noqa: E402
import concourse.bacc as bacc  # noqa: E402
import concourse.tile as tile  # noqa: E402
from concourse import mybir  # noqa: E402

B = 2
T = 2048
D = 1024
H = 16
HD = 64
G = 4  # heads per core
NCORES = 8
NT = T // 128  # 16 t-tiles
ND = D // 128  # 8 d-chunks
EPS = 1e-6
ROPE_BASE = 10000.0
MASK_NEG = -30000.0

F32 = mybir.dt.float32
BF16 = mybir.dt.bfloat16
AX = mybir.AxisListType
ALU = mybir.AluOpType
ACTF = mybir.ActivationFunctionType


def _rot_view(base_ap, col0, nseg):
    """View of base_ap cols [col0, col0+64*nseg) with halves swapped per
    64-wide segment: (P, nseg, 2, 32) reading [32:64] then [0:32]."""
    pstep = base_ap.ap[0]
    estep = base_ap.ap[-1][0]
    return bass.AP(
        tensor=base_ap.tensor,
        offset=base_ap.offset + (col0 + 32) * estep,
        ap=[pstep, [64 * estep, nseg], [-32 * estep, 2], [estep, 32]],
    )


def _seg_view(base_ap, col0, nseg):
    """(P, nseg, 64) view of base_ap cols [col0, col0+64*nseg)."""
    pstep = base_ap.ap[0]
    estep = base_ap.ap[-1][0]
    return bass.AP(
        tensor=base_ap.tensor,
        offset=base_ap.offset + col0 * estep,
        ap=[pstep, [64 * estep, nseg], [estep, 64]],
    )


def build_program():
    nc = bacc.Bacc(None, target_bir_lowering=False, debug=False)

    with tile.TileContext(nc) as tc:
        ctx = ExitStack()
        with ctx:
            dram = ctx.enter_context(tc.tile_pool(name="dram", bufs=1, space="DRAM"))
            xT_d = dram.tile([ND, 128, T], BF16, kind="ExternalInput", name="xT", uniquify=False)
            wqkv_d = dram.tile([ND, 128, 772], BF16, kind="ExternalInput", name="wqkv", uniquify=False)
            wo_d = dram.tile([2, 128, D], BF16, kind="ExternalInput", name="wo", uniquify=False)
            ctab_d = dram.tile([NT, 128, 128], BF16, kind="ExternalInput", name="ctab", uniquify=False)
            stab_d = dram.tile([NT, 128, 128], BF16, kind="ExternalInput", name="stab", uniquify=False)
            mtri_d = dram.tile([128, 128], F32, kind="ExternalInput", name="mtri", uniquify=False)
            ident_d = dram.tile([128, 128], F32, kind="ExternalInput", name="ident", uniquify=False)
            vones_d = dram.tile([128, NT, G], BF16, kind="ExternalInput", name="vones", uniquify=False)
            outp_d = dram.tile([NT, 128, D], F32, kind="ExternalOutput", name="outp", uniquify=False)

            # ---- persistent SBUF (whole kernel) ----
            persist = ctx.enter_context(tc.tile_pool(name="persist", bufs=1))
            v_all = persist.tile([128, NT, 260], BF16)     # V' natural, 4x(64+ones)
            qt01 = persist.tile([128, T], BF16)            # Q^T heads 0,1 stacked
            qt23 = persist.tile([128, T], BF16)
            kt01 = persist.tile([128, T], BF16)
            kt23 = persist.tile([128, T], BF16)
            mtri_s = persist.tile([128, 128], F32)
            stats_all = persist.tile([128, NT, 8], F32)
            rv_all = persist.tile([128, NT, 8], F32)
            eps_s = persist.tile([128, 1], F32)
            ident_s = persist.tile([128, 128], F32)
            nc.vector.memset(eps_s, EPS)

            nc.sync.dma_start(out=mtri_s, in_=mtri_d)
            nc.sync.dma_start(out=ident_s, in_=ident_d)

            # ================= Phase 1: projections + rmsnorm + rope + transpose
            p1 = ExitStack()
            with p1:
                xpool = p1.enter_context(tc.tile_pool(name="xpool", bufs=1))
                wpool = p1.enter_context(tc.tile_pool(name="wpool", bufs=1))
                tabpool = p1.enter_context(tc.tile_pool(name="tabpool", bufs=1))
                work1 = p1.enter_context(tc.tile_pool(name="work1", bufs=2))
                ps_qk = p1.enter_context(tc.tile_pool(name="ps_qk", bufs=2, space="PSUM"))
                ps_v = p1.enter_context(tc.tile_pool(name="ps_v", bufs=2, space="PSUM"))
                ps_tr = p1.enter_context(tc.tile_pool(name="ps_tr", bufs=1, space="PSUM"))

                w_s = wpool.tile([128, ND, 772], BF16)
                xt_s = []
                for c in range(ND):
                    xc = xpool.tile([128, T], BF16, name=f"xt{c}")
                    nc.sync.dma_start(out=xc, in_=xT_d[c])
                    nc.sync.dma_start(out=w_s[:, c, :], in_=wqkv_d[c])
                    xt_s.append(xc)
                ctab_s = tabpool.tile([128, NT, 128], BF16)
                stab_s = tabpool.tile([128, NT, 128], BF16)
                nc.sync.dma_start(out=ctab_s, in_=ctab_d.rearrange("t p n -> p t n"))
                nc.sync.dma_start(out=stab_s, in_=stab_d.rearrange("t p n -> p t n"))

                vones_cols = bass.AP(
                    tensor=v_all.tensor,
                    offset=v_all.offset + 64,
                    ap=[v_all.ap[0], [260, NT], [65, G]])
                nc.sync.dma_start(out=vones_cols, in_=vones_d)

                dsts = (qt01, qt23, kt01, kt23)
                for r4 in range(NT // 4):  # rounds of 4 t-tiles
                    trp = [ps_tr.tile([128, 512], F32, name=f"tr{cb}", tag=f"tr{cb}")
                           for cb in range(4)]
                    for it in range(4 * r4, 4 * r4 + 4):
                        qkp = ps_qk.tile([128, 512], F32, tag="qk")
                        vp = ps_v.tile([128, 260], F32, tag="v")
                        for c in range(ND):
                            lhs = xt_s[c][:, it * 128:(it + 1) * 128]
                            nc.tensor.matmul(qkp, lhs, w_s[:, c, 0:512],
                                             start=(c == 0), stop=(c == ND - 1))
                            nc.tensor.matmul(vp, lhs, w_s[:, c, 512:772],
                                             start=(c == 0), stop=(c == ND - 1))
                        # rms stats: squares on ACT (PSUM read 1), sums on DVE
                        scr = work1.tile([128, 512], BF16, tag="scr")
                        nc.scalar.activation(out=scr, in_=qkp, func=ACTF.Square)
                        # V drain early in ACT queue (vp bank release path)
                        vdst = bass.AP(
                            tensor=v_all.tensor,
                            offset=v_all[:, it, :].offset,
                            ap=[v_all.ap[0], [65, 4], [1, 64]])
                        vsrc = bass.AP(
                            tensor=vp.tensor,
                            offset=vp.offset,
                            ap=[vp.ap[0], [65, 4], [1, 64]])
                        nc.scalar.copy(vdst, vsrc)
                        nc.vector.tensor_reduce(
                            out=stats_all[:, it, :],
                            in_=scr.rearrange("p (s e) -> p s e", e=64),
                            axis=AX.X, op=ALU.add)
                        # rv = 1/sqrt(sumsq/64 + eps): ACT sqrt + fast recip
                        nc.scalar.activation(out=stats_all[:, it, :],
                                             in_=stats_all[:, it, :],
                                             func=ACTF.Sqrt,
                                             scale=1.0 / HD, bias=eps_s)
                        with nc.allow_low_precision(reason="rms scale recip"):
                            nc.vector.reciprocal_approx_fast(
                                out=rv_all[:, it, :], in_=stats_all[:, it, :])
                        # fused rms apply: qhat = qkp * rv, PSUM read 2
                        qhat = work1.tile([128, 512], BF16, tag="qhat")
                        rvb = bass.AP(
                            tensor=rv_all.tensor,
                            offset=rv_all[:, it, :].offset,
                            ap=[rv_all.ap[0], [1, 8], [0, 64]])
                        nc.vector.tensor_tensor(
                            out=qhat.rearrange("p (s e) -> p s e", e=64),
                            in0=qkp.rearrange("p (s e) -> p s e", e=64),
                            in1=rvb, op=ALU.mult)
                        # rope: t1 = qhat*c + rot(qhat)*s (tables have q|k halves)
                        t1 = work1.tile([128, 512], F32, tag="t1")
                        rot = work1.tile([128, 512], F32, tag="rot")
                        cb_view = bass.AP(
                            tensor=ctab_s.tensor,
                            offset=ctab_s[:, it, :].offset,
                            ap=[ctab_s.ap[0], [64, 2], [0, 4], [1, 64]])
                        nc.vector.tensor_tensor(
                            out=t1.rearrange("p (h r e) -> p h r e", h=2, r=4),
                            in0=qhat.rearrange("p (h r e) -> p h r e", h=2, r=4),
                            in1=cb_view, op=ALU.mult)
                        for half, c0 in ((0, 0), (1, 256)):
                            sb_view = bass.AP(
                                tensor=stab_s.tensor,
                                offset=stab_s[:, it, :].offset + 64 * half,
                                ap=[stab_s.ap[0], [0, 4], [32, 2], [1, 32]])
                            rot_out = bass.AP(
                                tensor=rot.tensor,
                                offset=rot.offset + c0,
                                ap=[rot.ap[0], [64, 4], [32, 2], [1, 32]])
                            nc.vector.tensor_tensor(
                                out=rot_out, in0=_rot_view(qhat, c0, 4),
                                in1=sb_view, op=ALU.mult)
                        nc.vector.tensor_add(out=t1, in0=t1, in1=rot)
                        # transpose 4 col-blocks via PE (bf16, 1 cyc/row)
                        for cb in range(4):
                            nc.tensor.transpose(
                                trp[cb][:, (it % 4) * 128:(it % 4 + 1) * 128],
                                t1[:, cb * 128:(cb + 1) * 128], ident_s)
                    for cb in range(4):
                        nc.vector.tensor_copy(
                            dsts[cb][:, r4 * 512:(r4 + 1) * 512], trp[cb])

            # ====== Phases 2+3 fused: pair-packed attention + oproj bursts
            p23 = ExitStack()
            with p23:
                otpool = p23.enter_context(tc.tile_pool(name="otpool", bufs=1))
                # heads stacked per pair: rows 0-63 = even head, 64-127 = odd
                ot01 = otpool.tile([128, T], BF16)
                ot23 = otpool.tile([128, T], BF16)
                wo_s = otpool.tile([128, 2, D], BF16)  # [hv-pair rows, pair, D]
                nc.sync.dma_start(out=wo_s, in_=wo_d.rearrange("a p n -> p a n"))

                ptpool = p23.enter_context(tc.tile_pool(name="ptpool", bufs=3))
                npool = p23.enter_context(tc.tile_pool(name="npool", bufs=2))
                outpool = p23.enter_context(tc.tile_pool(name="outpool", bufs=3))
                ps_sg = p23.enter_context(tc.tile_pool(name="ps_sg", bufs=2, space="PSUM"))
                ps_o = p23.enter_context(tc.tile_pool(name="ps_o", bufs=2, space="PSUM"))

                def oproj_block(jb):
                    for it in range(4 * jb, 4 * jb + 4):
                        pso = ps_sg.tile([128, 1024], F32, tag="sg")
                        for n in range(2):
                            psl = slice(n * 512, (n + 1) * 512)
                            nc.tensor.matmul(
                                pso[:, psl],
                                ot01[:, it * 128:(it + 1) * 128],
                                wo_s[:, 0, n * 512:(n + 1) * 512],
                                start=True, stop=False)
                            nc.tensor.matmul(
                                pso[:, psl],
                                ot23[:, it * 128:(it + 1) * 128],
                                wo_s[:, 1, n * 512:(n + 1) * 512],
                                start=False, stop=True)
                        osb = outpool.tile([128, D], F32, tag="osb")
                        nc.scalar.copy(osb, pso)
                        nc.sync.dma_start(out=outp_d[it], in_=osb)

                pending = []  # deferred normalize muls: (op, rec64, ott, half, j)

                def flush_pending():
                    for op_, rec64_, ott_, half_, j_ in pending:
                        nc.vector.tensor_mul(
                            out=ott_[64 * half_:64 * (half_ + 1),
                                     j_ * 512:(j_ + 1) * 512],
                            in0=op_[0:64, 0:512], in1=rec64_)
                    pending.clear()

                for j in range(4):
                    kmax = 4 * (j + 1)
                    for pair in range(2):
                        qt = (qt01, qt23)[pair]
                        kt = (kt01, kt23)[pair]
                        ott = (ot01, ot23)[pair]
                        opA = ps_o.tile([65, 512], F32, tag="opA")
                        opB = ps_o.tile([65, 512], F32, tag="opB")
                        for i in range(kmax):
                            if i == 1:
                                flush_pending()
                            sg = ps_sg.tile([128, 1024], F32, tag="sg")
                            pt = ptpool.tile([128, 1024], BF16, tag="pt")
                            nc.tensor.matmul(
                                sg[:, 0:512],
                                kt[0:64, i * 128:(i + 1) * 128],
                                qt[0:64, j * 512:(j + 1) * 512],
                                start=True, stop=True)
                            nc.tensor.matmul(
                                sg[:, 512:1024],
                                kt[64:128, i * 128:(i + 1) * 128],
                                qt[64:128, j * 512:(j + 1) * 512],
                                start=True, stop=True)
                            r = i - 4 * j
                            c0 = max(0, 128 * r) if r >= 0 else 0
                            if r >= 0:  # diagonal block mask, both heads
                                for half in range(2):
                                    dsl = slice(half * 512 + 128 * r,
                                                half * 512 + 128 * (r + 1))
                                    nc.vector.tensor_add(
                                        out=sg[:, dsl], in0=sg[:, dsl], in1=mtri_s)
                            nc.scalar.activation(out=pt, in_=sg, func=ACTF.Exp)
                            nc.tensor.matmul(
                                opA[:, c0:512],
                                v_all[:, i, (2 * pair) * 65:(2 * pair + 1) * 65],
                                pt[:, c0:512],
                                start=(i == 0), stop=(i == kmax - 1))
                            nc.tensor.matmul(
                                opB[:, c0:512],
                                v_all[:, i, (2 * pair + 1) * 65:(2 * pair + 2) * 65],
                                pt[:, 512 + c0:1024],
                                start=(i == 0), stop=(i == kmax - 1))
                        # denominators -> recip -> DMA broadcast; mul deferred
                        for half, op in ((0, opA), (1, opB)):
                            den_sb = npool.tile([1, 512], F32, tag=f"den{half}")
                            nc.vector.tensor_copy(den_sb, op[64:65, 0:512])
                            rec_sb = npool.tile([1, 512], F32, tag=f"rec{half}")
                            with nc.allow_low_precision(reason="softmax den recip"):
                                nc.vector.reciprocal_approx_fast(
                                    out=rec_sb, in_=den_sb)
                            rec64 = npool.tile([64, 512], F32, tag=f"rec64{half}")
                            rec_b = bass.AP(
                                tensor=rec_sb.tensor,
                                offset=rec_sb.offset,
                                ap=[[1, 1], [0, 64], rec_sb.ap[-1]])
                            nc.sync.dma_start(out=rec64, in_=rec_b)
                            pending.append((op, rec64, ott, half, j))
                    # oproj deferred one j-phase so drains are long done
                    if j >= 1:
                        oproj_block(j - 1)
                flush_pending()
                oproj_block(3)

    nc.compile()
    return nc


_PROGRAM = None


def _get_program():
    global _PROGRAM
    if _PROGRAM is None:
        _PROGRAM = build_program()
    return _PROGRAM


def make_inputs_for_core(core, x, Wq, Wk, Wv, Wo, q_norm_w, k_norm_w):
    bf16 = ml_dtypes.bfloat16
    b, g = core // 4, core % 4
    xT = np.ascontiguousarray(np.asarray(x[b]).T).reshape(ND, 128, T)
    wq = np.asarray(Wq)[:, 256 * g:256 * (g + 1)]
    wk = np.asarray(Wk)[:, 256 * g:256 * (g + 1)]
    wv = np.asarray(Wv)[:, 256 * g:256 * (g + 1)]
    wvp = np.zeros((D, 260), np.float32)
    for h in range(G):
        wvp[:, h * 65:h * 65 + 64] = wv[:, h * 64:(h + 1) * 64]
    wqkv = np.concatenate([wq, wk, wvp], axis=1).reshape(ND, 128, 772)
    wo = np.ascontiguousarray(
        np.asarray(Wo)[256 * g:256 * (g + 1), :].reshape(2, 128, D))

    inv_freq = 1.0 / (ROPE_BASE ** (np.arange(0, HD, 2, dtype=np.float64) / HD))
    tarr = np.arange(T, dtype=np.float64)
    fr = np.outer(tarr, inv_freq)
    cos, sin = np.cos(fr), np.sin(fr)

    def tables(w, scale):
        c = np.empty((T, HD), np.float64)
        s = np.empty((T, HD), np.float64)
        c[:, :32] = cos * w[:32] * scale
        c[:, 32:] = cos * w[32:] * scale
        s[:, :32] = -sin * w[32:] * scale
        s[:, 32:] = sin * w[:32] * scale
        return c, s

    qw = np.asarray(q_norm_w, np.float64)
    kw = np.asarray(k_norm_w, np.float64)
    cq, sq = tables(qw, 0.125)
    ck, sk = tables(kw, 1.0)
    ctab = np.concatenate([cq, ck], axis=1).reshape(NT, 128, 128)
    stab = np.concatenate([sq, sk], axis=1).reshape(NT, 128, 128)

    kp = np.arange(128)[:, None]
    qf = np.arange(128)[None, :]
    mtri = np.where(qf >= kp, 0.0, MASK_NEG).astype(np.float32)
    vones = np.ones((128, NT, G), bf16)
    ident = np.eye(128).astype(np.float32)
    return {
        "xT": xT.astype(bf16), "wqkv": wqkv.astype(bf16), "wo": wo.astype(bf16),
        "ctab": ctab.astype(bf16), "stab": stab.astype(bf16),
        "mtri": mtri, "vones": vones, "ident": ident,
    }


def run_on_hw(inputs, trace=False):
    from concourse.bass_utils import run_bass_kernel_spmd
    nc = _get_program()
    in_maps = [make_inputs_for_core(c, **inputs) for c in range(NCORES)]
    res = run_bass_kernel_spmd(nc, in_maps, list(range(NCORES)), trace=trace)
    parts = [res.results[c]["outp"].reshape(T, D) for c in range(NCORES)]
    out = np.stack([sum(parts[0:4]), sum(parts[4:8])]).astype(np.float32)
    return out, res


def kernel(**inputs):
    out, _ = run_on_hw(inputs, trace=False)
    return out
